# revision 1
# baseline (speedup 1.0000x reference)
"""Trainium2 Bass kernel for an AttnBlock (GroupNorm -> QKV 1x1 conv ->
spatial self-attention -> output projection -> residual).

Full-input contract: kernel(**inputs) takes the unsharded numpy inputs and
returns the full (4, 512, 64, 64) float32 output.

Sharding: 8 cores = 4 batches x 2 query-halves. Each core group-norms its
batch, runs attention for its 2048 queries over all 4096 keys, and writes
its query-half of the output. The per-core x input is column-rotated on the
host so that each core's own queries are always columns [0, 2048) — this
keeps the SPMD program identical across cores.

Algebraic fusions (all exact up to rounding):
- scores: q_i.k_j = h_j^T (Wk^T Wq) h_i + (Wk^T bq).h_j + [terms constant
  in j, dropped: softmax over j is invariant]. So K is never materialized;
  S^T = H^T @ R with R = (Wk^T Wq)^T-weighted H_q, and the (Wk^T bq).h_j
  term enters as a per-partition bias of the exp activation.
- attention output: Wp @ (V P) = (Wp Wv) @ (H P) + Wp bv (softmax weights
  sum to 1), so V is never materialized either: A = H-space attention
  (lhsT = H^T blocks), projected by M2 = Wp Wv, plus w4 = Wp bv + bp.
- softmax skips the max-subtraction; a constant -4.0 folded into the exp
  bias guards fp8e4m3 overflow (cancels exactly in the normalization).
  Denominators: E tiles are accumulated on DVE and reduced across
  partitions by a single all-ones fp32 matmul per query chunk, and divided
  out after the output projection.

Numerics: score-side matmuls in bf16; the attention-value matmuls run in
fp8e4 with perf_mode=DoubleRow (two key sub-rows per PE cell, K=256 per
matmul); everything accumulates in fp32 PSUM, and statistics, softmax
denominators and the final combine stay fp32.
"""

from contextlib import ExitStack

import numpy as np

import concourse.mybir as mybir
import concourse.tile as tile
from concourse import bacc
from concourse.bass_utils import run_bass_kernel_spmd

# Problem geometry (hardcoded; the grading harness stages only kernel.py).
B = 4
C = 512
HW = 64
N = HW * HW          # 4096 keys per batch
NQ = N // 2          # 2048 queries per core
GROUPS = 32
GSIZE = C // GROUPS  # 16 channels per group
EPS = 1e-6

P = 128
CT = C // P          # 4 channel chunks
JT = N // P          # 32 key chunks of 128
NI = 512             # free-dim tile (queries / keys / channels)
IC = NQ // NI        # 4 query chunks per core

F32 = mybir.dt.float32
BF16 = mybir.dt.bfloat16

PARAM_NAMES = ("bq", "bk", "bv", "bp", "gn_scale", "gn_bias")
WEIGHT_NAMES = ("wq", "wk", "wv", "wp")

_BUILD_CACHE = {}


def _emit(ctx, nc, tc, x_d, w_d, p_d, out_d, repeat=1):
    AF = mybir.ActivationFunctionType
    ALU = mybir.AluOpType

    consts = ctx.enter_context(tc.tile_pool(name="consts", bufs=1))
    small = ctx.enter_context(tc.tile_pool(name="small", bufs=4))
    stage = ctx.enter_context(tc.tile_pool(name="stage", bufs=6))
    big = ctx.enter_context(tc.tile_pool(name="big", bufs=2))
    rpool = ctx.enter_context(tc.tile_pool(name="rpool", bufs=1))
    wpool = ctx.enter_context(tc.tile_pool(name="wpool", bufs=1))
    epool = ctx.enter_context(tc.tile_pool(name="epool", bufs=6))
    attn_pool = ctx.enter_context(tc.tile_pool(name="attn_pool", bufs=2))
    outs_pool = ctx.enter_context(tc.tile_pool(name="outs_pool", bufs=3))
    mm_ps = ctx.enter_context(tc.tile_pool(name="mm_ps", bufs=4, space="PSUM"))
    acc_ps = ctx.enter_context(tc.tile_pool(name="acc_ps", bufs=4, space="PSUM"))

    for _rep in range(repeat):
        _emit_body(nc, tc, x_d, w_d, p_d, out_d, consts, small, stage, big,
                   rpool, wpool, epool, attn_pool, outs_pool, mm_ps, acc_ps,
                   AF, ALU, _rep)


def _emit_body(nc, tc, x_d, w_d, p_d, out_d, consts, small, stage, big,
               rpool, wpool, epool, attn_pool, outs_pool, mm_ps, acc_ps,
               AF, ALU, rep):
    # ---- constants -------------------------------------------------------
    # Pool-engine constants first: the hT transposes need `ident_bf` and
    # nothing should queue ahead of it on GpSimd.
    ident_bf = consts.tile([P, P], BF16, tag="ident_bf")
    nc.gpsimd.memset(ident_bf, 0.0)
    nc.gpsimd.affine_select(
        out=ident_bf, in_=ident_bf, compare_op=ALU.not_equal, fill=1.0,
        base=0, pattern=[[-1, P]], channel_multiplier=1,
    )
    ones_f = consts.tile([P, P], F32, tag="ones_f")
    nc.vector.memset(ones_f, 1.0)

    # Per-channel params as (128, CT): column cc = channels [cc*128, ..+128).
    # SWDGE (gpsimd) keeps these small gathers off the HWDGE queues that
    # stream x and the weights.
    par = {}
    for name in PARAM_NAMES:
        t = consts.tile([P, CT], F32, tag=f"par_{name}", name=f"par_{name}")
        nc.gpsimd.dma_start(out=t, in_=p_d[name][:].rearrange("(t p) -> p t", p=P))
        par[name] = t
    # Group-reduction matrices. G: (128, 8) with G[p, g] = 1/GSIZE iff
    # p // GSIZE == g. GE: (8, 128) with GE[g, p] = 1 iff p // GSIZE == g.
    GPC = P // GSIZE  # 8 groups per 128-channel chunk
    gmat = consts.tile([P, GPC], F32, tag="gmat")
    nc.gpsimd.memset(gmat, 1.0 / GSIZE)
    nc.gpsimd.affine_select(
        out=gmat, in_=gmat, compare_op=ALU.is_ge, fill=0.0,
        base=0, pattern=[[-GSIZE, GPC]], channel_multiplier=1,
    )
    nc.gpsimd.affine_select(
        out=gmat, in_=gmat, compare_op=ALU.is_ge, fill=0.0,
        base=GSIZE - 1, pattern=[[GSIZE, GPC]], channel_multiplier=-1,
    )
    gexp = consts.tile([GPC, P], F32, tag="gexp")
    nc.gpsimd.memset(gexp, 1.0)
    nc.gpsimd.affine_select(
        out=gexp, in_=gexp, compare_op=ALU.is_ge, fill=0.0,
        base=0, pattern=[[1, P]], channel_multiplier=-GSIZE,
    )
    nc.gpsimd.affine_select(
        out=gexp, in_=gexp, compare_op=ALU.is_ge, fill=0.0,
        base=GSIZE - 1, pattern=[[-1, P]], channel_multiplier=GSIZE,
    )
    eps8 = consts.tile([GPC, 1], F32, tag="eps8")
    nc.vector.memset(eps8, EPS)

    # ---- weights: one DMA + one bf16 cast per weight --------------------
    # The host ships "wp" already transposed (c_in on rows), so all four
    # arrive in the layout their matmuls need.
    w_nat = {}
    for wname in WEIGHT_NAMES:
        w_nat[wname] = wpool.tile([P, CT, C], BF16, tag=f"wn_{wname}",
                                  name=f"wn_{wname}")
        ws = stage.tile([P, CT, C], F32, tag="wstage",
                        name=f"ws_{rep}_{wname}", bufs=2)
        nc.sync.dma_start(
            out=ws, in_=w_d[wname][:].rearrange("(t p) c -> p t c", p=P))
        nc.vector.tensor_copy(out=w_nat[wname], in_=ws)
    wpT = w_nat["wp"]
    # bf16 bias casts (only needed by the w2/w4 fusions below)
    bq_bf = consts.tile([P, CT], BF16, tag="bq_bf")
    nc.vector.tensor_copy(out=bq_bf, in_=par["bq"])
    bv_bf = consts.tile([P, CT], BF16, tag="bv_bf")
    nc.vector.tensor_copy(out=bv_bf, in_=par["bv"])

    # ---- weight-only fusions (overlap with the x DMA / GroupNorm) --------
    # W3 = Wq^T Wk, stored (b=c_q partition-chunks, a=c_k free).
    w3 = wpool.tile([P, CT, C], BF16, tag="w3")
    for bt in range(CT):
        ps = mm_ps.tile([P, C], F32, tag="mm")
        for co in range(CT):
            nc.tensor.matmul(
                ps, lhsT=w_nat["wq"][:, co, bt * P:(bt + 1) * P],
                rhs=w_nat["wk"][:, co, :],
                start=(co == 0), stop=(co == CT - 1))
        nc.vector.tensor_copy(out=w3[:, bt, :], in_=ps)
    # M2T = (Wp Wv)^T, stored (a=c_attn partition-chunks, d=c_out free).
    m2t = wpool.tile([P, CT, C], BF16, tag="m2t")
    for at in range(CT):
        ps = mm_ps.tile([P, C], F32, tag="mm")
        for ec in range(CT):
            nc.tensor.matmul(
                ps, lhsT=w_nat["wv"][:, ec, at * P:(at + 1) * P],
                rhs=wpT[:, ec, :],
                start=(ec == 0), stop=(ec == CT - 1))
        nc.vector.tensor_copy(out=m2t[:, at, :], in_=ps)
    # w2 = Wk^T bq (bf16, used as a matmul operand against h).
    w2_bf = consts.tile([P, CT], BF16, tag="w2_bf")
    for at in range(CT):
        ps = mm_ps.tile([P, 1], F32, tag="mm")
        for co in range(CT):
            nc.tensor.matmul(
                ps, lhsT=w_nat["wk"][:, co, at * P:(at + 1) * P],
                rhs=bq_bf[:, co:co + 1],
                start=(co == 0), stop=(co == CT - 1))
        nc.vector.tensor_copy(out=w2_bf[:, at:at + 1], in_=ps)
    # w4 = Wp bv + bp (per output channel, f32).
    w4 = consts.tile([P, CT], F32, tag="w4")
    for dt_ in range(CT):
        ps = mm_ps.tile([P, 1], F32, tag="mm")
        for ec in range(CT):
            nc.tensor.matmul(
                ps, lhsT=wpT[:, ec, dt_ * P:(dt_ + 1) * P],
                rhs=bv_bf[:, ec:ec + 1],
                start=(ec == 0), stop=(ec == CT - 1))
        nc.vector.tensor_add(out=w4[:, dt_:dt_ + 1], in0=ps,
                             in1=par["bp"][:, dt_:dt_ + 1])

    # ---- x load + GroupNorm + normalize (to bf16 h) ----------------------
    h = big.tile([P, CT, N], BF16, tag="big")
    # hT blocks (keys on partitions), filled per channel chunk as h lands.
    ht = big.tile([P, JT, C], mybir.dt.float8e4, tag="big")
    for cc in range(CT):
        stats = small.tile([P, 8, 6], F32, tag="gn_stats",
                           name=f"gn_stats_{rep}_{cc}")
        xs = stage.tile([P, N], F32, tag="xstage", name=f"xs_{rep}_{cc}",
                        bufs=2)
        nc.sync.dma_start(out=xs, in_=x_d[cc * P:(cc + 1) * P, :])
        for sg in range(8):
            nc.vector.bn_stats(out=stats[:, sg, :],
                               in_=xs[:, sg * NI:(sg + 1) * NI])
        mv = small.tile([P, 2], F32, tag="gn_mv")
        nc.vector.bn_aggr(out=mv, in_=stats)
        # stat2 = [mean_c, E[x^2]_c];  E[x^2] = mean^2 + var in one op
        stat2 = small.tile([P, 2], F32, tag="gn_stat2")
        nc.vector.tensor_copy(out=stat2[:, 0:1], in_=mv[:, 0:1])
        nc.vector.tensor_scalar(
            out=stat2[:, 1:2], in0=mv[:, 0:1], scalar1=mv[:, 0:1],
            scalar2=mv[:, 1:2], op0=ALU.mult, op1=ALU.add)
        # group-combine on PE: (8, 2) = G^T @ stat2
        g_ps = acc_ps.tile([GPC, 2], F32, tag="acc")
        nc.tensor.matmul(g_ps, lhsT=gmat, rhs=stat2, start=True, stop=True)
        g_sb = small.tile([GPC, 2], F32, tag="gn_gsb")
        nc.vector.tensor_copy(out=g_sb, in_=g_ps)
        # grp = [mean_g, rstd_g];  rstd via sqrt(-1*(mean^2 - E2) + eps)
        grp = small.tile([GPC, 2], F32, tag="gn_grp")
        nc.vector.tensor_copy(out=grp[:, 0:1], in_=g_sb[:, 0:1])
        nvar = small.tile([GPC, 1], F32, tag="gn_nvar")
        nc.vector.tensor_scalar(
            out=nvar, in0=g_sb[:, 0:1], scalar1=g_sb[:, 0:1],
            scalar2=g_sb[:, 1:2], op0=ALU.mult, op1=ALU.subtract)
        sd = small.tile([GPC, 1], F32, tag="gn_sd")
        nc.scalar.activation(out=sd, in_=nvar, func=AF.Sqrt, bias=eps8,
                             scale=-1.0)
        nc.vector.reciprocal(out=grp[:, 1:2], in_=sd)
        # expand back to per-channel via PE: (128, 2) = GE^T @ grp
        e_ps = acc_ps.tile([P, 2], F32, tag="acc")
        nc.tensor.matmul(e_ps, lhsT=gexp, rhs=grp, start=True, stop=True)
        e_sb = small.tile([P, 2], F32, tag="gn_esb")
        nc.vector.tensor_copy(out=e_sb, in_=e_ps)
        # a_c = gn_scale * rstd ; b_c = gn_bias - mean * a_c
        a_c = small.tile([P, 1], F32, tag="gn_a")
        nc.vector.tensor_mul(out=a_c, in0=par["gn_scale"][:, cc:cc + 1],
                             in1=e_sb[:, 1:2])
        nb_c = small.tile([P, 1], F32, tag="gn_nb")
        nc.vector.tensor_scalar(
            out=nb_c, in0=e_sb[:, 0:1], scalar1=a_c,
            scalar2=par["gn_bias"][:, cc:cc + 1],
            op0=ALU.mult, op1=ALU.subtract)
        # b_c for the ACT half (needs the true sign)
        b_c = small.tile([P, 1], F32, tag="gn_b")
        nc.vector.tensor_scalar_mul(out=b_c, in0=nb_c, scalar1=-1.0)
        # h = a_c * x - nb_c, split across DVE and ACT halves
        nc.vector.tensor_scalar(
            out=h[:, cc, :N // 2], in0=xs[:, :N // 2], scalar1=a_c,
            scalar2=nb_c, op0=ALU.mult, op1=ALU.subtract)
        nc.scalar.activation(
            out=h[:, cc, N // 2:], in_=xs[:, N // 2:], func=AF.Identity,
            scale=a_c, bias=b_c)
        # hT blocks for this channel chunk: 4 transposes packed per PSUM
        # bank (disjoint column ranges), one strided eviction per pack.
        for jg in range(JT // 4):
            tp = acc_ps.tile([P, 4, P], BF16, tag="acc",
                             name=f"htp_{rep}_{cc}_{jg}")
            for k in range(4):
                jc = jg * 4 + k
                nc.tensor.matmul(
                    tp[:, k, :], lhsT=h[:, cc, jc * P:(jc + 1) * P],
                    rhs=ident_bf, is_transpose=True, skip_group_check=True)
            dst = ht[:, jg * 4:(jg + 1) * 4, cc * P:(cc + 1) * P]
            if jg % 2 == 0:
                nc.vector.tensor_copy(out=dst, in_=tp)
            else:
                nc.scalar.activation(out=dst, in_=tp, func=AF.Identity)

    # ---- h-derived operands ---------------------------------------------
    inv_sqrt_c = float(C) ** -0.5
    # R = (Wk^T Wq)^T-weighted H_q: R[a, i] = sum_b W3[b, a] h[b, i].
    # icq-major so attention on the first query chunk can start early.
    r_sb = rpool.tile([P, CT, NQ], BF16, tag="r")
    for icq in range(IC):
        for at in range(CT):
            ps = mm_ps.tile([P, NI], F32, tag="mm")
            for bc in range(CT):
                nc.tensor.matmul(
                    ps, lhsT=w3[:, bc, at * P:(at + 1) * P],
                    rhs=h[:, bc, icq * NI:(icq + 1) * NI],
                    start=(bc == 0), stop=(bc == CT - 1))
            nc.vector.tensor_copy(out=r_sb[:, at, icq * NI:(icq + 1) * NI],
                                  in_=ps)
    # r2[j] = (Wk^T bq) . h_j, scaled by c^-0.5: per-partition exp bias.
    # 8 j-chunks pack into one PSUM bank (disjoint f32 columns).
    r2s = consts.tile([P, JT], F32, tag="r2s")
    for jg in range(JT // 8):
        ps = acc_ps.tile([P, 8], F32, tag="acc", name=f"r2p_{rep}_{jg}")
        for k in range(8):
            jc = jg * 8 + k
            for ac in range(CT):
                nc.tensor.matmul(
                    ps[:, k:k + 1], lhsT=h[:, ac, jc * P:(jc + 1) * P],
                    rhs=w2_bf[:, ac:ac + 1],
                    start=(ac == 0), stop=(ac == CT - 1),
                    skip_group_check=True)
        # -4.0 guards fp8e4m3 exp overflow (448 max); the e^-4 factor
        # cancels exactly in the softmax normalization.
        nc.vector.tensor_scalar(out=r2s[:, jg * 8:(jg + 1) * 8], in0=ps,
                                scalar1=inv_sqrt_c, scalar2=-4.0,
                                op0=ALU.mult, op1=ALU.add)

    # ---- attention + output projection + residual ------------------------
    for icq in range(IC):
        att_ps = [acc_ps.tile([P, NI], F32, tag="acc",
                              name=f"att_ps_{rep}_{icq}_{ct}")
                  for ct in range(CT)]
        e_sum = outs_pool.tile([P, NI], F32, tag="esum", bufs=2,
                                name=f"esum_{rep}_{icq}")
        for jp in range(JT // 2):
            e2 = epool.tile([P, 2, NI], mybir.dt.float8e4, tag="e",
                            name=f"e2_{rep}_{icq}_{jp}")
            for half in range(2):
                jc = jp * 2 + half
                s_ps = mm_ps.tile([P, NI], F32, tag="mm",
                                  name=f"s_ps_{rep}_{icq}_{jc}")
                for ac in range(CT):
                    nc.tensor.matmul(
                        s_ps, lhsT=h[:, ac, jc * P:(jc + 1) * P],
                        rhs=r_sb[:, ac, icq * NI:(icq + 1) * NI],
                        start=(ac == 0), stop=(ac == CT - 1))
                nc.scalar.activation(out=e2[:, half, :], in_=s_ps,
                                     func=AF.Exp, scale=inv_sqrt_c,
                                     bias=r2s[:, jc:jc + 1])
            for ct in range(CT):
                nc.tensor.matmul(
                    att_ps[ct], lhsT=ht[:, 2 * jp:2 * jp + 2,
                                        ct * P:(ct + 1) * P],
                    rhs=e2, start=(jp == 0), stop=(jp == JT // 2 - 1),
                    perf_mode=mybir.MatmulPerfMode.DoubleRow)
            if jp == 0:
                nc.vector.tensor_copy(out=e_sum, in_=e2[:, 0, :])
            else:
                nc.vector.tensor_add(out=e_sum, in0=e_sum, in1=e2[:, 0, :])
            nc.vector.tensor_add(out=e_sum, in0=e_sum, in1=e2[:, 1, :])
        den_ps = mm_ps.tile([P, NI], F32, tag="mm",
                            name=f"den_ps_{rep}_{icq}")
        nc.tensor.matmul(den_ps, lhsT=ones_f, rhs=e_sum, start=True, stop=True)
        rec = outs_pool.tile([P, NI], F32, tag="rec", bufs=2,
                              name=f"rec_{rep}_{icq}")
        nc.vector.reciprocal(out=rec, in_=den_ps)
        att_sb = attn_pool.tile([P, CT, NI], BF16, tag="attn")
        for ct in range(CT):
            nc.vector.tensor_copy(out=att_sb[:, ct, :], in_=att_ps[ct])
        xr = outs_pool.tile([P, CT, NI], F32, tag="xres", bufs=2,
                            name=f"xr_{rep}_{icq}")
        nc.sync.dma_start(
            out=xr, in_=x_d[:, icq * NI:(icq + 1) * NI].rearrange(
                "(t p) n -> p t n", p=P))
        for dc in range(CT):
            pp = mm_ps.tile([P, NI], F32, tag="mm")
            for ct in range(CT):
                nc.tensor.matmul(
                    pp, lhsT=m2t[:, ct, dc * P:(dc + 1) * P],
                    rhs=att_sb[:, ct, :],
                    start=(ct == 0), stop=(ct == CT - 1))
            ob = outs_pool.tile([P, NI], F32, tag="ob")
            nc.vector.tensor_mul(out=ob, in0=pp, in1=rec)
            nc.vector.tensor_scalar_add(out=ob, in0=ob,
                                        scalar1=w4[:, dc:dc + 1])
            nc.vector.tensor_add(out=ob, in0=ob, in1=xr[:, dc, :])
            nc.sync.dma_start(
                out=out_d[dc * P:(dc + 1) * P, icq * NI:(icq + 1) * NI], in_=ob)


def _build(repeat=1):
    nc = bacc.Bacc()
    x_d = nc.declare_dram_parameter("x", [C, N], F32, isOutput=False)
    w_d = {w: nc.declare_dram_parameter(w, [C, C], F32, isOutput=False)
           for w in WEIGHT_NAMES}
    p_d = {p: nc.declare_dram_parameter(p, [C], F32, isOutput=False)
           for p in PARAM_NAMES}
    out_d = nc.declare_dram_parameter("out", [C, NQ], F32, isOutput=True)
    with tile.TileContext(nc) as tc, ExitStack() as ctx:
        _emit(ctx, nc, tc, x_d, w_d, p_d, out_d, repeat=repeat)
    nc.finalize()
    return nc


def _get_nc():
    if "nc" not in _BUILD_CACHE:
        _BUILD_CACHE["nc"] = _build()
    return _BUILD_CACHE["nc"]


def _make_in_maps(x, gn_scale, gn_bias, wq, bq, wk, bk, wv, bv, wp, bp):
    xf = np.ascontiguousarray(np.asarray(x, dtype=np.float32).reshape(B, C, N))
    shared = {
        "wq": np.ascontiguousarray(np.asarray(wq, np.float32)),
        "wk": np.ascontiguousarray(np.asarray(wk, np.float32)),
        "wv": np.ascontiguousarray(np.asarray(wv, np.float32)),
        # wp ships pre-transposed: the kernel wants c_in on rows.
        "wp": np.ascontiguousarray(np.asarray(wp, np.float32).T),
        "bq": np.ascontiguousarray(np.asarray(bq, np.float32)),
        "bk": np.ascontiguousarray(np.asarray(bk, np.float32)),
        "bv": np.ascontiguousarray(np.asarray(bv, np.float32)),
        "bp": np.ascontiguousarray(np.asarray(bp, np.float32)),
        "gn_scale": np.ascontiguousarray(np.asarray(gn_scale, np.float32)),
        "gn_bias": np.ascontiguousarray(np.asarray(gn_bias, np.float32)),
    }
    in_maps = []
    for core in range(8):
        bi, qh = core // 2, core % 2
        xb = xf[bi]
        if qh == 0:
            xc = xb
        else:
            xc = np.ascontiguousarray(
                np.concatenate([xb[:, NQ:], xb[:, :NQ]], axis=1))
        in_maps.append({"x": xc, **shared})
    return in_maps


def _gather(results):
    out = np.empty((B, C, N), np.float32)
    for core in range(8):
        bi, qh = core // 2, core % 2
        out[bi, :, qh * NQ:(qh + 1) * NQ] = results[core]["out"]
    return out.reshape(B, C, HW, HW)


def kernel(x, gn_scale, gn_bias, wq, bq, wk, bk, wv, bv, wp, bp):
    nc = _get_nc()
    in_maps = _make_in_maps(x, gn_scale, gn_bias, wq, bq, wk, bk, wv, bv,
                            wp, bp)
    res = run_bass_kernel_spmd(nc, in_maps, core_ids=list(range(8)))
    return _gather(res.results)



# revision 3
# speedup vs baseline: 1.0066x; 1.0066x over previous
"""Trainium2 Bass kernel for an AttnBlock (GroupNorm -> QKV 1x1 conv ->
spatial self-attention -> output projection -> residual).

Full-input contract: kernel(**inputs) takes the unsharded numpy inputs and
returns the full (4, 512, 64, 64) float32 output.

Sharding: 8 cores = 4 batches x 2 query-halves. Each core group-norms its
batch, runs attention for its 2048 queries over all 4096 keys, and writes
its query-half of the output. The per-core x input is column-rotated on the
host so that each core's own queries are always columns [0, 2048) — this
keeps the SPMD program identical across cores.

Algebraic fusions (all exact up to rounding):
- scores: q_i.k_j = h_j^T (Wk^T Wq) h_i + (Wk^T bq).h_j + [terms constant
  in j, dropped: softmax over j is invariant]. So K is never materialized;
  S = H^T @ R with R = W3^T H, W3 = Wq^T Wk, and the (Wk^T bq).h_j term
  enters as a per-partition bias of the exp activation.
- attention output: Wp @ (V P) = (Wp Wv) @ (H P) + Wp bv (softmax weights
  sum to 1), so V is never materialized either, and the output projection
  collapses into M2 = Wp Wv, plus w4 = Wp bv + bp.
- W3, M2^T, w2 = Wk^T bq and w4 are precomputed on the HOST (f32 numpy)
  and shipped pre-quantized: W3/M2T scaled by 16 into fp8e4m3 (their
  entries are ~N(0, 1/c), x16 centers them in fp8 range), w2 in fp8, w4 in
  f32. This removes all on-chip weight prep and 4MB/core of weight DMA.
- softmax skips the max-subtraction; a constant -4.0 folded into the exp
  bias guards fp8e4m3 overflow (cancels exactly in the normalization).
  Denominators come straight from the PE: an all-ones fp8 DoubleRow matmul
  accumulates sum_j e2[j, i] alongside the attention matmuls (bit-identical
  to summing the same fp8 e2 tiles on DVE, but ~10x cheaper).

Numerics: h is stored fp8e4m3 only (GroupNorm output is ~N(0,1)); every
main-loop matmul (scores, R, attention-value, projection) runs in fp8 with
perf_mode=DoubleRow (two contraction sub-rows per PE cell, K=256 per
matmul); everything accumulates in fp32 PSUM. The attention accumulator is
divided by the softmax denominator BEFORE the output projection, so its
values live in the convex hull of h and re-quantize safely to fp8.
"""

from contextlib import ExitStack

import numpy as np

import concourse.mybir as mybir
import concourse.tile as tile
from concourse import bacc
from concourse.bass_utils import run_bass_kernel_spmd

# Problem geometry (hardcoded; the grading harness stages only kernel.py).
B = 4
C = 512
HW = 64
N = HW * HW          # 4096 keys per batch
NQ = N // 2          # 2048 queries per core
GROUPS = 32
GSIZE = C // GROUPS  # 16 channels per group
EPS = 1e-6
WSCALE = 16.0        # host-side scale on W3 / M2T before fp8 quantization

P = 128
CT = C // P          # 4 channel chunks
CP = CT // 2         # 2 channel chunk-pairs (fp8 DoubleRow)
JT = N // P          # 32 key chunks of 128
NI = 512             # free-dim tile (queries / keys / channels)
IC = NQ // NI        # 4 query chunks per core

F32 = mybir.dt.float32
BF16 = mybir.dt.bfloat16
F8 = mybir.dt.float8e4

PARAM_NAMES = ("gn_scale", "gn_bias", "w4")

_BUILD_CACHE = {}


def _emit(ctx, nc, tc, x_d, w3_d, m2_d, w2_d, p_d, out_d, repeat=1):
    AF = mybir.ActivationFunctionType
    ALU = mybir.AluOpType

    consts = ctx.enter_context(tc.tile_pool(name="consts", bufs=1))
    small = ctx.enter_context(tc.tile_pool(name="small", bufs=4))
    stage = ctx.enter_context(tc.tile_pool(name="stage", bufs=2))
    big = ctx.enter_context(tc.tile_pool(name="big", bufs=2))
    rpool = ctx.enter_context(tc.tile_pool(name="rpool", bufs=1))
    wpool = ctx.enter_context(tc.tile_pool(name="wpool", bufs=1))
    epool = ctx.enter_context(tc.tile_pool(name="epool", bufs=6))
    attn_pool = ctx.enter_context(tc.tile_pool(name="attn_pool", bufs=2))
    outs_pool = ctx.enter_context(tc.tile_pool(name="outs_pool", bufs=3))
    mm_ps = ctx.enter_context(tc.tile_pool(name="mm_ps", bufs=3, space="PSUM"))
    acc_ps = ctx.enter_context(tc.tile_pool(name="acc_ps", bufs=4, space="PSUM"))

    for _rep in range(repeat):
        _emit_body(nc, tc, x_d, w3_d, m2_d, w2_d, p_d, out_d, consts, small,
                   stage, big, rpool, wpool, epool, attn_pool, outs_pool,
                   mm_ps, acc_ps, AF, ALU, _rep)


def _emit_body(nc, tc, x_d, w3_d, m2_d, w2_d, p_d, out_d, consts, small,
               stage, big, rpool, wpool, epool, attn_pool, outs_pool,
               mm_ps, acc_ps, AF, ALU, rep):
    # ---- constants -------------------------------------------------------
    # Pool-engine identity first (bf16, proven path), then a one-time DVE
    # cast to the fp8 identity the h-transposes use.
    ident_bf = consts.tile([P, P], BF16, tag="ident_bf")
    nc.gpsimd.memset(ident_bf, 0.0)
    nc.gpsimd.affine_select(
        out=ident_bf, in_=ident_bf, compare_op=ALU.not_equal, fill=1.0,
        base=0, pattern=[[-1, P]], channel_multiplier=1,
    )
    ident_f8 = consts.tile([P, P], F8, tag="ident_f8")
    nc.vector.tensor_copy(out=ident_f8, in_=ident_bf)
    # all-ones fp8 DoubleRow operand for the softmax denominators
    ones8 = consts.tile([P, 2, P], F8, tag="ones8")
    nc.vector.memset(ones8, 1.0)

    # Per-channel params as (128, CT): column cc = channels [cc*128, ..+128).
    # SWDGE (gpsimd) keeps these small gathers off the HWDGE queue that
    # streams x.
    par = {}
    for name in PARAM_NAMES:
        t = consts.tile([P, CT], F32, tag=f"par_{name}", name=f"par_{name}")
        nc.gpsimd.dma_start(out=t, in_=p_d[name][:].rearrange("(t p) -> p t", p=P))
        par[name] = t
    w2t = consts.tile([P, CT], F8, tag="w2t")
    nc.gpsimd.dma_start(out=w2t, in_=w2_d[:].rearrange("(t p) -> p t", p=P))
    # Host-fused weights (already x16 fp8). Also on SWDGE.
    w3t = wpool.tile([P, CT, C], F8, tag="w3t")
    nc.gpsimd.dma_start(out=w3t, in_=w3_d[:].rearrange("(t p) c -> p t c", p=P))
    m2t = wpool.tile([P, CT, C], F8, tag="m2t")
    nc.gpsimd.dma_start(out=m2t, in_=m2_d[:].rearrange("(t p) c -> p t c", p=P))

    # Group-reduction matrices. G: (128, 8) with G[p, g] = 1/GSIZE iff
    # p // GSIZE == g. GE: (8, 128) with GE[g, p] = 1 iff p // GSIZE == g.
    GPC = P // GSIZE  # 8 groups per 128-channel chunk
    gmat = consts.tile([P, GPC], F32, tag="gmat")
    nc.gpsimd.memset(gmat, 1.0 / GSIZE)
    nc.gpsimd.affine_select(
        out=gmat, in_=gmat, compare_op=ALU.is_ge, fill=0.0,
        base=0, pattern=[[-GSIZE, GPC]], channel_multiplier=1,
    )
    nc.gpsimd.affine_select(
        out=gmat, in_=gmat, compare_op=ALU.is_ge, fill=0.0,
        base=GSIZE - 1, pattern=[[GSIZE, GPC]], channel_multiplier=-1,
    )
    gexp = consts.tile([GPC, P], F32, tag="gexp")
    nc.gpsimd.memset(gexp, 1.0)
    nc.gpsimd.affine_select(
        out=gexp, in_=gexp, compare_op=ALU.is_ge, fill=0.0,
        base=0, pattern=[[1, P]], channel_multiplier=-GSIZE,
    )
    nc.gpsimd.affine_select(
        out=gexp, in_=gexp, compare_op=ALU.is_ge, fill=0.0,
        base=GSIZE - 1, pattern=[[-1, P]], channel_multiplier=GSIZE,
    )
    eps8 = consts.tile([GPC, 1], F32, tag="eps8")
    nc.vector.memset(eps8, EPS)

    # ---- x load + GroupNorm + normalize (straight to fp8 h) --------------
    h8 = big.tile([P, CT, N], F8, tag="big")
    # hT blocks (keys on partitions), filled per channel chunk as h lands.
    ht = big.tile([P, JT, C], F8, tag="big")
    for cc in range(CT):
        stats = small.tile([P, 8, 6], F32, tag="gn_stats",
                           name=f"gn_stats_{rep}_{cc}")
        xs = stage.tile([P, N], F32, tag="xstage", name=f"xs_{rep}_{cc}",
                        bufs=2)
        nc.sync.dma_start(out=xs, in_=x_d[cc * P:(cc + 1) * P, :])
        for sg in range(8):
            nc.vector.bn_stats(out=stats[:, sg, :],
                               in_=xs[:, sg * NI:(sg + 1) * NI])
        mv = small.tile([P, 2], F32, tag="gn_mv")
        nc.vector.bn_aggr(out=mv, in_=stats)
        # stat2 = [mean_c, E[x^2]_c];  E[x^2] = mean^2 + var in one op
        stat2 = small.tile([P, 2], F32, tag="gn_stat2")
        nc.vector.tensor_copy(out=stat2[:, 0:1], in_=mv[:, 0:1])
        nc.vector.tensor_scalar(
            out=stat2[:, 1:2], in0=mv[:, 0:1], scalar1=mv[:, 0:1],
            scalar2=mv[:, 1:2], op0=ALU.mult, op1=ALU.add)
        # group-combine on PE: (8, 2) = G^T @ stat2
        g_ps = acc_ps.tile([GPC, 2], F32, tag="acc")
        nc.tensor.matmul(g_ps, lhsT=gmat, rhs=stat2, start=True, stop=True)
        g_sb = small.tile([GPC, 2], F32, tag="gn_gsb")
        nc.vector.tensor_copy(out=g_sb, in_=g_ps)
        # grp = [mean_g, rstd_g];  rstd via sqrt(-1*(mean^2 - E2) + eps)
        grp = small.tile([GPC, 2], F32, tag="gn_grp")
        nc.vector.tensor_copy(out=grp[:, 0:1], in_=g_sb[:, 0:1])
        nvar = small.tile([GPC, 1], F32, tag="gn_nvar")
        nc.vector.tensor_scalar(
            out=nvar, in0=g_sb[:, 0:1], scalar1=g_sb[:, 0:1],
            scalar2=g_sb[:, 1:2], op0=ALU.mult, op1=ALU.subtract)
        sd = small.tile([GPC, 1], F32, tag="gn_sd")
        nc.scalar.activation(out=sd, in_=nvar, func=AF.Sqrt, bias=eps8,
                             scale=-1.0)
        nc.vector.reciprocal(out=grp[:, 1:2], in_=sd)
        # expand back to per-channel via PE: (128, 2) = GE^T @ grp
        e_ps = acc_ps.tile([P, 2], F32, tag="acc")
        nc.tensor.matmul(e_ps, lhsT=gexp, rhs=grp, start=True, stop=True)
        e_sb = small.tile([P, 2], F32, tag="gn_esb")
        nc.vector.tensor_copy(out=e_sb, in_=e_ps)
        # a_c = gn_scale * rstd ; b_c = gn_bias - mean * a_c
        a_c = small.tile([P, 1], F32, tag="gn_a")
        nc.vector.tensor_mul(out=a_c, in0=par["gn_scale"][:, cc:cc + 1],
                             in1=e_sb[:, 1:2])
        nb_c = small.tile([P, 1], F32, tag="gn_nb")
        nc.vector.tensor_scalar(
            out=nb_c, in0=e_sb[:, 0:1], scalar1=a_c,
            scalar2=par["gn_bias"][:, cc:cc + 1],
            op0=ALU.mult, op1=ALU.subtract)
        b_c = small.tile([P, 1], F32, tag="gn_b")
        nc.vector.tensor_scalar_mul(out=b_c, in0=nb_c, scalar1=-1.0)
        # h8 = a_c * x + b_c on ACT (single pass; DVE keeps the stats work)
        nc.scalar.activation(
            out=h8[:, cc, :], in_=xs, func=AF.Identity,
            scale=a_c, bias=b_c)
        # hT blocks for this channel chunk: 4 fp8 transposes packed per PSUM
        # bank (disjoint column ranges), one strided eviction per pack.
        # FP8 transpose writes with an element step of 2 in PSUM, so the
        # pack is allocated 2x wide and accessed with stride 2.
        for jg in range(JT // 4):
            tp = acc_ps.tile([P, 4, 2 * P], F8, tag="acc",
                             name=f"htp_{rep}_{cc}_{jg}")
            for k in range(4):
                jc = jg * 4 + k
                nc.tensor.matmul(
                    tp[:, k, 0:2 * P:2], lhsT=h8[:, cc, jc * P:(jc + 1) * P],
                    rhs=ident_f8, is_transpose=True, skip_group_check=True)
            dst = ht[:, jg * 4:(jg + 1) * 4, cc * P:(cc + 1) * P]
            src = tp[:, :, 0:2 * P:2]
            if jg % 2 == 0:
                nc.vector.tensor_copy(out=dst, in_=src)
            else:
                nc.scalar.activation(out=dst, in_=src, func=AF.Identity)

    # ---- h-derived operands ---------------------------------------------
    inv_sqrt_c = float(C) ** -0.5
    # r2[j] = (Wk^T bq) . h_j, scaled by c^-0.5: per-partition exp bias.
    # 8 j-chunks pack into one PSUM bank (disjoint f32 columns).
    r2s = consts.tile([P, JT], F32, tag="r2s")
    for jg in range(JT // 8):
        ps = acc_ps.tile([P, 8], F32, tag="acc", name=f"r2p_{rep}_{jg}")
        for k in range(8):
            jc = jg * 8 + k
            for ac in range(CT):
                nc.tensor.matmul(
                    ps[:, k:k + 1], lhsT=h8[:, ac, jc * P:(jc + 1) * P],
                    rhs=w2t[:, ac:ac + 1],
                    start=(ac == 0), stop=(ac == CT - 1),
                    skip_group_check=True)
        # -4.0 guards fp8e4m3 exp overflow; the e^-4 factor cancels exactly
        # in the softmax normalization.
        nc.vector.tensor_scalar(out=r2s[:, jg * 8:(jg + 1) * 8], in0=ps,
                                scalar1=inv_sqrt_c, scalar2=-4.0,
                                op0=ALU.mult, op1=ALU.add)
    # R = W3^T-weighted H_q: R[a, i] = sum_b W3[b, a] h[b, i]; fp8
    # DoubleRow over b chunk-pairs, evicted /16 back to fp8 natural scale.
    # icq-major so attention on the first query chunk can start early.
    r8 = rpool.tile([P, CT, NQ], F8, tag="r")
    for icq in range(IC):
        for at in range(CT):
            ps = mm_ps.tile([P, NI], F32, tag="mm")
            for bp_ in range(CP):
                nc.tensor.matmul(
                    ps, lhsT=w3t[:, 2 * bp_:2 * bp_ + 2, at * P:(at + 1) * P],
                    rhs=h8[:, 2 * bp_:2 * bp_ + 2, icq * NI:(icq + 1) * NI],
                    start=(bp_ == 0), stop=(bp_ == CP - 1),
                    perf_mode=mybir.MatmulPerfMode.DoubleRow)
            nc.vector.tensor_scalar_mul(
                out=r8[:, at, icq * NI:(icq + 1) * NI], in0=ps,
                scalar1=1.0 / WSCALE)

    # ---- attention + output projection + residual ------------------------
    for icq in range(IC):
        att_ps = [acc_ps.tile([P, NI], F32, tag="acc",
                              name=f"att_ps_{rep}_{icq}_{ct}")
                  for ct in range(CT)]
        den_ps = acc_ps.tile([P, NI], F32, tag="den", bufs=1,
                             name=f"den_ps_{rep}_{icq}")
        xr = outs_pool.tile([P, CT, NI], F32, tag="xres", bufs=2,
                            name=f"xr_{rep}_{icq}")
        nc.sync.dma_start(
            out=xr, in_=x_d[:, icq * NI:(icq + 1) * NI].rearrange(
                "(t p) n -> p t n", p=P))
        # xr4 = x + w4 per output-channel chunk, prepared on the idle Pool
        # engine while the jp loop runs.
        xr4 = outs_pool.tile([P, CT, NI], F32, tag="xres4", bufs=2,
                             name=f"xr4_{rep}_{icq}")
        for dc in range(CT):
            nc.gpsimd.tensor_scalar_add(out=xr4[:, dc, :], in0=xr[:, dc, :],
                                        scalar1=par["w4"][:, dc:dc + 1])
        for jp in range(JT // 2):
            e2 = epool.tile([P, 2, NI], F8, tag="e",
                            name=f"e2_{rep}_{icq}_{jp}")
            for half in range(2):
                jc = jp * 2 + half
                s_ps = mm_ps.tile([P, NI], F32, tag="mm",
                                  name=f"s_ps_{rep}_{icq}_{jc}")
                for ap_ in range(CP):
                    nc.tensor.matmul(
                        s_ps,
                        lhsT=h8[:, 2 * ap_:2 * ap_ + 2, jc * P:(jc + 1) * P],
                        rhs=r8[:, 2 * ap_:2 * ap_ + 2,
                               icq * NI:(icq + 1) * NI],
                        start=(ap_ == 0), stop=(ap_ == CP - 1),
                        perf_mode=mybir.MatmulPerfMode.DoubleRow)
                nc.scalar.activation(out=e2[:, half, :], in_=s_ps,
                                     func=AF.Exp, scale=inv_sqrt_c,
                                     bias=r2s[:, jc:jc + 1])
            for ct in range(CT):
                nc.tensor.matmul(
                    att_ps[ct], lhsT=ht[:, 2 * jp:2 * jp + 2,
                                        ct * P:(ct + 1) * P],
                    rhs=e2, start=(jp == 0), stop=(jp == JT // 2 - 1),
                    perf_mode=mybir.MatmulPerfMode.DoubleRow)
            nc.tensor.matmul(
                den_ps, lhsT=ones8, rhs=e2,
                start=(jp == 0), stop=(jp == JT // 2 - 1),
                perf_mode=mybir.MatmulPerfMode.DoubleRow)
        rec = outs_pool.tile([P, NI], F32, tag="rec", bufs=2,
                             name=f"rec_{rep}_{icq}")
        nc.vector.reciprocal(out=rec, in_=den_ps)
        # normalize the attention accumulator before the projection: values
        # land in the convex hull of h (|.| <~ 5), safe for fp8.
        att8 = attn_pool.tile([P, CT, NI], F8, tag="attn")
        for ct in range(CT):
            nc.vector.tensor_mul(out=att8[:, ct, :], in0=att_ps[ct], in1=rec)
        for dc in range(CT):
            pp = mm_ps.tile([P, NI], F32, tag="mm")
            for ep_ in range(CP):
                nc.tensor.matmul(
                    pp, lhsT=m2t[:, 2 * ep_:2 * ep_ + 2, dc * P:(dc + 1) * P],
                    rhs=att8[:, 2 * ep_:2 * ep_ + 2, :],
                    start=(ep_ == 0), stop=(ep_ == CP - 1),
                    perf_mode=mybir.MatmulPerfMode.DoubleRow)
            ob = outs_pool.tile([P, NI], F32, tag="ob")
            nc.vector.scalar_tensor_tensor(
                out=ob, in0=pp, scalar=1.0 / WSCALE, in1=xr4[:, dc, :],
                op0=ALU.mult, op1=ALU.add)
            nc.sync.dma_start(
                out=out_d[dc * P:(dc + 1) * P, icq * NI:(icq + 1) * NI], in_=ob)


def _build(repeat=1):
    nc = bacc.Bacc()
    x_d = nc.declare_dram_parameter("x", [C, N], F32, isOutput=False)
    w3_d = nc.declare_dram_parameter("w3", [C, C], F8, isOutput=False)
    m2_d = nc.declare_dram_parameter("m2t", [C, C], F8, isOutput=False)
    w2_d = nc.declare_dram_parameter("w2", [C], F8, isOutput=False)
    p_d = {p: nc.declare_dram_parameter(p, [C], F32, isOutput=False)
           for p in PARAM_NAMES}
    out_d = nc.declare_dram_parameter("out", [C, NQ], F32, isOutput=True)
    with tile.TileContext(nc) as tc, ExitStack() as ctx:
        _emit(ctx, nc, tc, x_d, w3_d, m2_d, w2_d, p_d, out_d, repeat=repeat)
    nc.finalize()
    return nc


def _get_nc():
    if "nc" not in _BUILD_CACHE:
        _BUILD_CACHE["nc"] = _build()
    return _BUILD_CACHE["nc"]


def _make_in_maps(x, gn_scale, gn_bias, wq, bq, wk, bk, wv, bv, wp, bp):
    f8np = mybir.dt.np(F8)
    xf = np.ascontiguousarray(np.asarray(x, dtype=np.float32).reshape(B, C, N))
    wqf = np.asarray(wq, np.float32)
    wkf = np.asarray(wk, np.float32)
    wvf = np.asarray(wv, np.float32)
    wpf = np.asarray(wp, np.float32)
    w3f = (wqf.T @ wkf) * WSCALE
    m2f = (wpf @ wvf).T * WSCALE
    w2f = wkf.T @ np.asarray(bq, np.float32)
    w4f = wpf @ np.asarray(bv, np.float32) + np.asarray(bp, np.float32)
    shared = {
        "w3": np.ascontiguousarray(w3f.astype(f8np)),
        "m2t": np.ascontiguousarray(m2f.astype(f8np)),
        "w2": np.ascontiguousarray(w2f.astype(f8np)),
        "w4": np.ascontiguousarray(w4f),
        "gn_scale": np.ascontiguousarray(np.asarray(gn_scale, np.float32)),
        "gn_bias": np.ascontiguousarray(np.asarray(gn_bias, np.float32)),
    }
    in_maps = []
    for core in range(8):
        bi, qh = core // 2, core % 2
        xb = xf[bi]
        if qh == 0:
            xc = xb
        else:
            xc = np.ascontiguousarray(
                np.concatenate([xb[:, NQ:], xb[:, :NQ]], axis=1))
        in_maps.append({"x": xc, **shared})
    return in_maps


def _gather(results):
    out = np.empty((B, C, N), np.float32)
    for core in range(8):
        bi, qh = core // 2, core % 2
        out[bi, :, qh * NQ:(qh + 1) * NQ] = results[core]["out"]
    return out.reshape(B, C, HW, HW)


def kernel(x, gn_scale, gn_bias, wq, bq, wk, bk, wv, bv, wp, bp):
    nc = _get_nc()
    in_maps = _make_in_maps(x, gn_scale, gn_bias, wq, bq, wk, bk, wv, bv,
                            wp, bp)
    res = run_bass_kernel_spmd(nc, in_maps, core_ids=list(range(8)))
    return _gather(res.results)


# revision 31
# speedup vs baseline: 1.1659x; 1.1583x over previous
"""Trainium2 Bass kernel for an AttnBlock (GroupNorm -> QKV 1x1 conv ->
spatial self-attention -> output projection -> residual).

Full-input contract: kernel(**inputs) takes the unsharded numpy inputs and
returns the full (4, 512, 64, 64) float32 output.

Sharding: 8 cores = 4 batches x 2 query-halves. Each core group-norms its
batch, runs attention for its 2048 queries over all 4096 keys, and writes
its query-half of the output. The per-core x input is column-rotated on the
host so that each core's own queries are always columns [0, 2048) — this
keeps the SPMD program identical across cores.

Algebraic fusions (all exact up to rounding):
- scores: q_i.k_j = h_j^T (Wk^T Wq) h_i + (Wk^T bq).h_j + [terms constant
  in j, dropped: softmax over j is invariant]. So K is never materialized;
  S = H^T @ R with R = W3^T H, W3 = Wq^T Wk, and the (Wk^T bq).h_j term
  enters as a per-partition bias of the exp activation.
- attention output: Wp @ (V P) = (Wp Wv) @ (H P) + Wp bv (softmax weights
  sum to 1), so V is never materialized either, and the output projection
  collapses into M2 = Wp Wv, plus w4 = Wp bv + bp.
- W3, M2^T, w2 = Wk^T bq and w4 are precomputed on the HOST (f32 numpy)
  and shipped pre-quantized: W3/M2T scaled by 16 into fp8e4m3 (their
  entries are ~N(0, 1/c), x16 centers them in fp8 range), w2 in fp8, w4 in
  f32. This removes all on-chip weight prep and 4MB/core of weight DMA.
- softmax skips the max-subtraction; a constant -4.0 folded into the exp
  bias guards fp8e4m3 overflow (cancels exactly in the normalization).
  Denominators come straight from the PE: an all-ones fp8 DoubleRow matmul
  accumulates sum_j e2[j, i] alongside the attention matmuls (bit-identical
  to summing the same fp8 e2 tiles on DVE, but ~10x cheaper).

Numerics: h is stored fp8e4m3 only (GroupNorm output is ~N(0,1)); every
main-loop matmul (scores, R, attention-value, projection) runs in fp8 with
perf_mode=DoubleRow (two contraction sub-rows per PE cell, K=256 per
matmul); everything accumulates in fp32 PSUM. The attention accumulator is
divided by the softmax denominator BEFORE the output projection, so its
values live in the convex hull of h and re-quantize safely to fp8.
"""

from contextlib import ExitStack

import numpy as np

import concourse.mybir as mybir
import concourse.tile as tile
from concourse import bacc
from concourse.bass_utils import run_bass_kernel_spmd

# Problem geometry (hardcoded; the grading harness stages only kernel.py).
B = 4
C = 512
HW = 64
N = HW * HW          # 4096 keys per batch
NQ = N // 2          # 2048 queries per core
GROUPS = 32
GSIZE = C // GROUPS  # 16 channels per group
EPS = 1e-6
WSCALE = 16.0        # host-side scale on W3 / M2T before fp8 quantization

P = 128
CT = C // P          # 4 channel chunks
CP = CT // 2         # 2 channel chunk-pairs (fp8 DoubleRow)
JT = N // P          # 32 key chunks of 128
NI = 512             # free-dim tile (queries / keys / channels)
IC = NQ // NI        # 4 query chunks per core

F32 = mybir.dt.float32
BF16 = mybir.dt.bfloat16
F8 = mybir.dt.float8e4

PARAM_NAMES = ("gn_scale", "gn_bias", "w4")

_BUILD_CACHE = {}


def _emit(ctx, nc, tc, x_d, x16_d, w3_d, m2_d, w2_d, p_d, out_d, repeat=1):
    AF = mybir.ActivationFunctionType
    ALU = mybir.AluOpType

    consts = ctx.enter_context(tc.tile_pool(name="consts", bufs=1))
    small = ctx.enter_context(tc.tile_pool(name="small", bufs=4))
    stage = ctx.enter_context(tc.tile_pool(name="stage", bufs=2))
    big = ctx.enter_context(tc.tile_pool(name="big", bufs=2))
    rpool = ctx.enter_context(tc.tile_pool(name="rpool", bufs=1))
    wpool = ctx.enter_context(tc.tile_pool(name="wpool", bufs=1))
    epool = ctx.enter_context(tc.tile_pool(name="epool", bufs=6))
    attn_pool = ctx.enter_context(tc.tile_pool(name="attn_pool", bufs=2))
    outs_pool = ctx.enter_context(tc.tile_pool(name="outs_pool", bufs=3))
    mm_ps = ctx.enter_context(tc.tile_pool(name="mm_ps", bufs=3, space="PSUM"))
    acc_ps = ctx.enter_context(tc.tile_pool(name="acc_ps", bufs=4, space="PSUM"))

    for _rep in range(repeat):
        _emit_body(nc, tc, x_d, x16_d, w3_d, m2_d, w2_d, p_d, out_d, consts,
                   small, stage, big, rpool, wpool, epool, attn_pool,
                   outs_pool, mm_ps, acc_ps, AF, ALU, _rep)


def _emit_body(nc, tc, x_d, x16_d, w3_d, m2_d, w2_d, p_d, out_d, consts,
               small, stage, big, rpool, wpool, epool, attn_pool, outs_pool,
               mm_ps, acc_ps, AF, ALU, rep):
    # ---- constants -------------------------------------------------------
    # Pool-engine identity first (bf16, proven path), then a one-time DVE
    # cast to the fp8 identity the h-transposes use.
    ident_bf = consts.tile([P, P], BF16, tag="ident_bf")
    nc.gpsimd.memset(ident_bf, 0.0)
    nc.gpsimd.affine_select(
        out=ident_bf, in_=ident_bf, compare_op=ALU.not_equal, fill=1.0,
        base=0, pattern=[[-1, P]], channel_multiplier=1,
    )
    ident_f8 = consts.tile([P, P], F8, tag="ident_f8")
    nc.vector.tensor_copy(out=ident_f8, in_=ident_bf)
    # all-ones fp8 DoubleRow operand for the softmax denominators
    ones8 = consts.tile([P, 2, P], F8, tag="ones8")
    nc.vector.memset(ones8, 1.0)

    # Per-channel params as (128, CT): column cc = channels [cc*128, ..+128).
    # SWDGE (gpsimd) keeps these small gathers off the HWDGE queue that
    # streams x.
    par = {}
    for name in PARAM_NAMES:
        t = consts.tile([P, CT], F32, tag=f"par_{name}", name=f"par_{name}")
        nc.gpsimd.dma_start(out=t, in_=p_d[name][:].rearrange("(t p) -> p t", p=P))
        par[name] = t
    w2t = consts.tile([P, CT], F8, tag="w2t")
    nc.gpsimd.dma_start(out=w2t, in_=w2_d[:].rearrange("(t p) -> p t", p=P))
    # Host-fused weights (already x16 fp8). Also on SWDGE.
    w3t = wpool.tile([P, CT, C], F8, tag="w3t")
    nc.gpsimd.dma_start(out=w3t, in_=w3_d[:].rearrange("(t p) c -> p t c", p=P))
    m2t = wpool.tile([P, CT, C], F8, tag="m2t")
    nc.gpsimd.dma_start(out=m2t, in_=m2_d[:].rearrange("(t p) c -> p t c", p=P))

    # Group-reduction matrices. G: (128, 8) with G[p, g] = 1/GSIZE iff
    # p // GSIZE == g. GE: (8, 128) with GE[g, p] = 1 iff p // GSIZE == g.
    GPC = P // GSIZE  # 8 groups per 128-channel chunk
    gmat = consts.tile([P, GPC], F32, tag="gmat")
    nc.gpsimd.memset(gmat, 1.0 / GSIZE)
    nc.gpsimd.affine_select(
        out=gmat, in_=gmat, compare_op=ALU.is_ge, fill=0.0,
        base=0, pattern=[[-GSIZE, GPC]], channel_multiplier=1,
    )
    nc.gpsimd.affine_select(
        out=gmat, in_=gmat, compare_op=ALU.is_ge, fill=0.0,
        base=GSIZE - 1, pattern=[[GSIZE, GPC]], channel_multiplier=-1,
    )
    gexp = consts.tile([GPC, P], F32, tag="gexp")
    nc.gpsimd.memset(gexp, 1.0)
    nc.gpsimd.affine_select(
        out=gexp, in_=gexp, compare_op=ALU.is_ge, fill=0.0,
        base=0, pattern=[[1, P]], channel_multiplier=-GSIZE,
    )
    nc.gpsimd.affine_select(
        out=gexp, in_=gexp, compare_op=ALU.is_ge, fill=0.0,
        base=GSIZE - 1, pattern=[[-1, P]], channel_multiplier=GSIZE,
    )
    eps8 = consts.tile([GPC, 1], F32, tag="eps8")
    nc.vector.memset(eps8, EPS)

    # ---- x load + GroupNorm + normalize (straight to fp8 h) --------------
    h8 = big.tile([P, CT, N], F8, tag="big")
    # hT blocks (keys on partitions), filled per channel chunk as h lands.
    ht = big.tile([P, JT, C], F8, tag="big")
    for cc in range(CT):
        stats = small.tile([P, 8, 6], F32, tag="gn_stats",
                           name=f"gn_stats_{rep}_{cc}")
        # GroupNorm reads the host-provided bf16 copy of x (half the DMA
        # bytes; the f32 original is still used for the residual). Two
        # half-chunk DMAs so the stats for the first half overlap the
        # second half's transfer.
        xs = stage.tile([P, N], BF16, tag="xstage", name=f"xs_{rep}_{cc}",
                        bufs=4)
        for hh in range(2):
            nc.sync.dma_start(
                out=xs[:, hh * (N // 2):(hh + 1) * (N // 2)],
                in_=x16_d[cc * P:(cc + 1) * P,
                          hh * (N // 2):(hh + 1) * (N // 2)])
            for sg in range(4):
                s4 = hh * 4 + sg
                nc.vector.bn_stats(out=stats[:, s4, :],
                                   in_=xs[:, s4 * NI:(s4 + 1) * NI])
        mv = small.tile([P, 2], F32, tag="gn_mv")
        nc.vector.bn_aggr(out=mv, in_=stats)
        # stat2 = [mean_c, E[x^2]_c];  E[x^2] = mean^2 + var in one op
        stat2 = small.tile([P, 2], F32, tag="gn_stat2")
        nc.vector.tensor_copy(out=stat2[:, 0:1], in_=mv[:, 0:1])
        nc.vector.tensor_scalar(
            out=stat2[:, 1:2], in0=mv[:, 0:1], scalar1=mv[:, 0:1],
            scalar2=mv[:, 1:2], op0=ALU.mult, op1=ALU.add)
        # group-combine on PE: (8, 2) = G^T @ stat2
        g_ps = acc_ps.tile([GPC, 2], F32, tag="acc")
        nc.tensor.matmul(g_ps, lhsT=gmat, rhs=stat2, start=True, stop=True)
        g_sb = small.tile([GPC, 2], F32, tag="gn_gsb")
        nc.vector.tensor_copy(out=g_sb, in_=g_ps)
        # grp = [mean_g, rstd_g];  rstd via sqrt(-1*(mean^2 - E2) + eps)
        grp = small.tile([GPC, 2], F32, tag="gn_grp")
        nc.vector.tensor_copy(out=grp[:, 0:1], in_=g_sb[:, 0:1])
        nvar = small.tile([GPC, 1], F32, tag="gn_nvar")
        nc.vector.tensor_scalar(
            out=nvar, in0=g_sb[:, 0:1], scalar1=g_sb[:, 0:1],
            scalar2=g_sb[:, 1:2], op0=ALU.mult, op1=ALU.subtract)
        sd = small.tile([GPC, 1], F32, tag="gn_sd")
        nc.scalar.activation(out=sd, in_=nvar, func=AF.Sqrt, bias=eps8,
                             scale=-1.0)
        nc.vector.reciprocal(out=grp[:, 1:2], in_=sd)
        # expand back to per-channel via PE: (128, 2) = GE^T @ grp
        e_ps = acc_ps.tile([P, 2], F32, tag="acc")
        nc.tensor.matmul(e_ps, lhsT=gexp, rhs=grp, start=True, stop=True)
        e_sb = small.tile([P, 2], F32, tag="gn_esb")
        nc.vector.tensor_copy(out=e_sb, in_=e_ps)
        # a_c = gn_scale * rstd ; b_c = gn_bias - mean * a_c
        a_c = small.tile([P, 1], F32, tag="gn_a")
        nc.vector.tensor_mul(out=a_c, in0=par["gn_scale"][:, cc:cc + 1],
                             in1=e_sb[:, 1:2])
        nb_c = small.tile([P, 1], F32, tag="gn_nb")
        nc.vector.tensor_scalar(
            out=nb_c, in0=e_sb[:, 0:1], scalar1=a_c,
            scalar2=par["gn_bias"][:, cc:cc + 1],
            op0=ALU.mult, op1=ALU.subtract)
        b_c = small.tile([P, 1], F32, tag="gn_b")
        nc.vector.tensor_scalar_mul(out=b_c, in0=nb_c, scalar1=-1.0)
        # h8 = a_c * x + b_c on ACT (DVE keeps the stats work). The last
        # chunk is normalized in two pieces so R / the first scores can
        # start as soon as the leading columns are ready.
        if cc < CT - 1:
            nc.scalar.activation(
                out=h8[:, cc, :], in_=xs, func=AF.Identity,
                scale=a_c, bias=b_c)
        else:
            nc.scalar.activation(
                out=h8[:, cc, :2 * NI], in_=xs[:, :2 * NI],
                func=AF.Identity, scale=a_c, bias=b_c)
            last_norm = (xs, a_c, b_c)

    inv_sqrt_c = float(C) ** -0.5

    def emit_transposes(cc, dve_only, jgs=None):
        # hT blocks for channel chunk cc: 4 fp8 transposes packed per PSUM
        # bank (disjoint column ranges), one strided eviction per pack.
        # FP8 transpose writes with an element step of 2 in PSUM, so the
        # pack is allocated 2x wide and accessed with stride 2. Evictions
        # go mostly to the otherwise-idle Pool engine; for the last chunk
        # they split Pool/DVE so ACT can start the exps immediately.
        for jg in (range(JT // 4) if jgs is None else jgs):
            tp = acc_ps.tile([P, 4, 2 * P], F8, tag="acc",
                             name=f"htp_{rep}_{cc}_{jg}")
            for k in range(4):
                jc = jg * 4 + k
                nc.tensor.matmul(
                    tp[:, k, 0:2 * P:2], lhsT=h8[:, cc, jc * P:(jc + 1) * P],
                    rhs=ident_f8, is_transpose=True, skip_group_check=True)
            dst = ht[:, jg * 4:(jg + 1) * 4, cc * P:(cc + 1) * P]
            src = tp[:, :, 0:2 * P:2]
            # GPSIMD cannot read PSUM on HW, so evictions split ACT/DVE:
            # ACT-heavy for the GroupNorm chunks (DVE paces the stats),
            # DVE-only for the last chunk (ACT must start the exps).
            if dve_only:
                eng = "dve"
            else:
                eng = ("act", "act", "dve", "act")[jg % 4]
            if eng == "dve":
                nc.vector.tensor_copy(out=dst, in_=src)
            else:
                nc.scalar.activation(out=dst, in_=src, func=AF.Identity)

    # R = W3^T-weighted H_q: R[a, i] = sum_b W3[b, a] h[b, i]; fp8
    # DoubleRow over b chunk-pairs, evicted /16 back to fp8 natural scale.
    r8 = rpool.tile([P, CT, NQ], F8, tag="r")

    def emit_r2(icq, ats):
        for at in ats:
            ps = mm_ps.tile([P, NI], F32, tag="mm",
                            name=f"rps_{rep}_{icq}_{at}")
            for bp_ in range(CP):
                nc.tensor.matmul(
                    ps, lhsT=w3t[:, 2 * bp_:2 * bp_ + 2, at * P:(at + 1) * P],
                    rhs=h8[:, 2 * bp_:2 * bp_ + 2, icq * NI:(icq + 1) * NI],
                    start=(bp_ == 0), stop=(bp_ == CP - 1),
                    perf_mode=mybir.MatmulPerfMode.DoubleRow)
            nc.vector.tensor_scalar_mul(
                out=r8[:, at, icq * NI:(icq + 1) * NI], in0=ps,
                scalar1=1.0 / WSCALE)

    def emit_r2s(jgs):
        # r2[j] = (Wk^T bq) . h_j, scaled by c^-0.5: per-partition exp bias.
        # 8 j-chunks pack into one PSUM bank (disjoint f32 columns).
        for jg in jgs:
            ps = acc_ps.tile([P, 8], F32, tag="acc", name=f"r2p_{rep}_{jg}")
            for k in range(8):
                jc = jg * 8 + k
                for ac in range(CT):
                    nc.tensor.matmul(
                        ps[:, k:k + 1], lhsT=h8[:, ac, jc * P:(jc + 1) * P],
                        rhs=w2t[:, ac:ac + 1],
                        start=(ac == 0), stop=(ac == CT - 1),
                        skip_group_check=True)
            # -4.0 guards fp8e4m3 exp overflow; the e^-4 factor cancels
            # exactly in the softmax normalization.
            nc.vector.tensor_scalar(out=r2s[:, jg * 8:(jg + 1) * 8], in0=ps,
                                    scalar1=inv_sqrt_c, scalar2=-4.0,
                                    op0=ALU.mult, op1=ALU.add)

    # Transposes for chunks 0..CT-2 already interleave with GroupNorm above;
    # for the last chunk, R(icq0) and the exp biases are emitted first so
    # the score pipeline can start as soon as h8 completes.
    r2s = consts.tile([P, JT], F32, tag="r2s")
    for cc in range(CT - 1):
        emit_transposes(cc, dve_only=False)
    # first-exp critical path: R(0) and the first exp biases / hT blocks
    # only need the leading 1024 columns of the last chunk (norm3a)
    emit_r2(0, range(CT))
    emit_r2s([0])
    emit_transposes(CT - 1, dve_only=True, jgs=[0, 1])
    xs3, a_c3, b_c3 = last_norm
    nc.scalar.activation(
        out=h8[:, CT - 1, 2 * NI:], in_=xs3[:, 2 * NI:],
        func=AF.Identity, scale=a_c3, bias=b_c3)
    emit_r2s([1, 2, 3])
    emit_transposes(CT - 1, dve_only=True, jgs=[2, 3, 4, 5, 6, 7])

    # ---- attention + output projection + residual ------------------------
    # Software-pipelined: the att/den matmuls trail the score matmuls by
    # ATT_LAG jp-steps so an in-order PE never head-blocks the ACT exp
    # stream; the epilogue of icq-1 is emitted inside icq's jp loop.
    ATT_LAG = 2
    JP = JT // 2

    def emit_attden(icq, att_ps, den_ps, jp, e2):
        # den first: it releases the reciprocal in the epilogue chain
        nc.tensor.matmul(
            den_ps, lhsT=ones8, rhs=e2,
            start=(jp == 0), stop=(jp == JP - 1),
            perf_mode=mybir.MatmulPerfMode.DoubleRow)
        for ct in range(CT):
            nc.tensor.matmul(
                att_ps[ct], lhsT=ht[:, 2 * jp:2 * jp + 2,
                                    ct * P:(ct + 1) * P],
                rhs=e2, start=(jp == 0), stop=(jp == JP - 1),
                perf_mode=mybir.MatmulPerfMode.DoubleRow)

    def emit_rec_evict(icq, att_ps, den_ps, rec, att8, cts, hs, do_rec):
        # rec + fp8 attention eviction, DVE/Pool alternated. Values land in
        # the convex hull of h (|.| <~ 5), safe for fp8.
        if do_rec:
            nc.vector.reciprocal(out=rec[:, hs], in_=den_ps[:, hs])
        for ct in cts:
            nc.vector.tensor_mul(out=att8[:, ct, hs],
                                 in0=att_ps[ct][:, hs], in1=rec[:, hs])

    def emit_proj(icq, att8, xr4, dc, hs, sp):
        NH = hs.stop - hs.start
        pp = mm_ps.tile([P, NH], F32, tag="mm",
                        name=f"pp_{rep}_{icq}_{sp}_{dc}")
        for ep_ in range(CP):
            nc.tensor.matmul(
                pp, lhsT=m2t[:, 2 * ep_:2 * ep_ + 2, dc * P:(dc + 1) * P],
                rhs=att8[:, 2 * ep_:2 * ep_ + 2, hs],
                start=(ep_ == 0), stop=(ep_ == CP - 1),
                perf_mode=mybir.MatmulPerfMode.DoubleRow)
        ob = outs_pool.tile([P, NH], F32, tag="ob",
                            name=f"ob_{rep}_{icq}_{sp}_{dc}")
        nc.vector.scalar_tensor_tensor(
            out=ob, in0=pp, scalar=1.0 / WSCALE,
            in1=xr4[:, dc, hs], op0=ALU.mult, op1=ALU.add)
        nc.sync.dma_start(
            out=out_d[dc * P:(dc + 1) * P,
                      icq * NI + hs.start:icq * NI + hs.stop],
            in_=ob)

    prev = None
    pst = None  # (rec, att8) of the in-flight epilogue
    lagq = []  # att/den matmuls trailing the score stream, across icq too
    FULL = slice(0, NI)
    for icq in range(IC):
        att_ps = [acc_ps.tile([P, NI], F32, tag="acc",
                              name=f"att_ps_{rep}_{icq}_{ct}")
                  for ct in range(CT)]
        den_ps = acc_ps.tile([P, NI], F32, tag="den", bufs=1,
                             name=f"den_ps_{rep}_{icq}")
        xr4 = None
        for jp in range(JP):
            # the previous query-chunk's epilogue, sliced thin across jp
            # slots so the injected PE work never outpaces the exp stream
            if prev is not None and 2 <= jp <= 8:
                picq, patt, pden, pxr4 = prev
                if jp == 2:
                    rec = outs_pool.tile([P, NI], F32, tag="rec", bufs=2,
                                         name=f"rec_{rep}_{picq}")
                    att8 = attn_pool.tile([P, CT, NI], F8, tag="attn",
                                          name=f"att8_{rep}_{picq}")
                    pst = (rec, att8)
                    emit_rec_evict(picq, patt, pden, rec, att8, (0, 1),
                                   FULL, do_rec=True)
                elif jp == 3:
                    emit_rec_evict(picq, patt, pden, pst[0], pst[1], (2, 3),
                                   FULL, do_rec=False)
                    if icq + 1 < IC:
                        emit_r2(icq + 1, (0, 1))
                elif jp == 4:
                    if icq + 1 < IC:
                        emit_r2(icq + 1, (2, 3))
                elif jp >= 5:
                    emit_proj(picq, pst[1], pxr4, jp - 5, FULL, 0)
                    if jp == 8:
                        prev = None
            elif prev is None and icq + 1 < IC and jp == 3:
                emit_r2(icq + 1, (0, 1))
            elif prev is None and icq + 1 < IC and jp == 4:
                emit_r2(icq + 1, (2, 3))
            if jp == 12:
                # residual load, emitted late so the scheduler cannot hoist
                # it ahead of the x chunk DMAs; w4 is folded in on Pool.
                xr = outs_pool.tile([P, CT, NI], F32, tag="xres", bufs=2,
                                    name=f"xr_{rep}_{icq}")
                nc.sync.dma_start(
                    out=xr, in_=x_d[:, icq * NI:(icq + 1) * NI].rearrange(
                        "(t p) n -> p t n", p=P))
                xr4 = outs_pool.tile([P, CT, NI], F32, tag="xres4", bufs=2,
                                     name=f"xr4_{rep}_{icq}")
                for dc in range(CT):
                    nc.gpsimd.tensor_scalar_add(
                        out=xr4[:, dc, :], in0=xr[:, dc, :],
                        scalar1=par["w4"][:, dc:dc + 1])
            e2 = epool.tile([P, 2, NI], F8, tag="e",
                            name=f"e2_{rep}_{icq}_{jp}")
            for half in range(2):
                jc = jp * 2 + half
                s_ps = mm_ps.tile([P, NI], F32, tag="mm",
                                  name=f"s_ps_{rep}_{icq}_{jc}")
                for ap_ in range(CP):
                    nc.tensor.matmul(
                        s_ps,
                        lhsT=h8[:, 2 * ap_:2 * ap_ + 2, jc * P:(jc + 1) * P],
                        rhs=r8[:, 2 * ap_:2 * ap_ + 2,
                               icq * NI:(icq + 1) * NI],
                        start=(ap_ == 0), stop=(ap_ == CP - 1),
                        perf_mode=mybir.MatmulPerfMode.DoubleRow)
                nc.scalar.activation(out=e2[:, half, :], in_=s_ps,
                                     func=AF.Exp, scale=inv_sqrt_c,
                                     bias=r2s[:, jc:jc + 1])
            lagq.append((icq, att_ps, den_ps, jp, e2))
            if len(lagq) > ATT_LAG:
                emit_attden(*lagq.pop(0))
        prev = (icq, att_ps, den_ps, xr4)
    while lagq:
        emit_attden(*lagq.pop(0))
    # tail epilogue, pipelined in two column halves
    licq, latt, lden, lxr4 = prev
    rec = outs_pool.tile([P, NI], F32, tag="rec", bufs=2,
                         name=f"rec_{rep}_{licq}")
    att8 = attn_pool.tile([P, CT, NI], F8, tag="attn",
                          name=f"att8_{rep}_{licq}")
    for sp in range(2):
        hs = slice(sp * (NI // 2), (sp + 1) * (NI // 2))
        emit_rec_evict(licq, latt, lden, rec, att8, range(CT), hs,
                       do_rec=True)
        for dc in range(CT):
            emit_proj(licq, att8, lxr4, dc, hs, sp)


def _build(repeat=1):
    nc = bacc.Bacc()
    x_d = nc.declare_dram_parameter("x", [C, N], F32, isOutput=False)
    x16_d = nc.declare_dram_parameter("x16", [C, N], BF16, isOutput=False)
    w3_d = nc.declare_dram_parameter("w3", [C, C], F8, isOutput=False)
    m2_d = nc.declare_dram_parameter("m2t", [C, C], F8, isOutput=False)
    w2_d = nc.declare_dram_parameter("w2", [C], F8, isOutput=False)
    p_d = {p: nc.declare_dram_parameter(p, [C], F32, isOutput=False)
           for p in PARAM_NAMES}
    out_d = nc.declare_dram_parameter("out", [C, NQ], F32, isOutput=True)
    with tile.TileContext(nc) as tc, ExitStack() as ctx:
        _emit(ctx, nc, tc, x_d, x16_d, w3_d, m2_d, w2_d, p_d, out_d,
              repeat=repeat)
    nc.finalize()
    return nc


def _get_nc():
    if "nc" not in _BUILD_CACHE:
        _BUILD_CACHE["nc"] = _build()
    return _BUILD_CACHE["nc"]


def _make_in_maps(x, gn_scale, gn_bias, wq, bq, wk, bk, wv, bv, wp, bp):
    f8np = mybir.dt.np(F8)
    xf = np.ascontiguousarray(np.asarray(x, dtype=np.float32).reshape(B, C, N))
    wqf = np.asarray(wq, np.float32)
    wkf = np.asarray(wk, np.float32)
    wvf = np.asarray(wv, np.float32)
    wpf = np.asarray(wp, np.float32)
    w3f = (wqf.T @ wkf) * WSCALE
    m2f = (wpf @ wvf).T * WSCALE
    w2f = wkf.T @ np.asarray(bq, np.float32)
    w4f = wpf @ np.asarray(bv, np.float32) + np.asarray(bp, np.float32)
    shared = {
        "w3": np.ascontiguousarray(w3f.astype(f8np)),
        "m2t": np.ascontiguousarray(m2f.astype(f8np)),
        "w2": np.ascontiguousarray(w2f.astype(f8np)),
        "w4": np.ascontiguousarray(w4f),
        "gn_scale": np.ascontiguousarray(np.asarray(gn_scale, np.float32)),
        "gn_bias": np.ascontiguousarray(np.asarray(gn_bias, np.float32)),
    }
    import ml_dtypes
    in_maps = []
    for core in range(8):
        bi, qh = core // 2, core % 2
        xb = xf[bi]
        if qh == 0:
            xc = xb
        else:
            xc = np.ascontiguousarray(
                np.concatenate([xb[:, NQ:], xb[:, :NQ]], axis=1))
        x16 = np.ascontiguousarray(xc.astype(ml_dtypes.bfloat16))
        in_maps.append({"x": xc, "x16": x16, **shared})
    return in_maps


def _gather(results):
    out = np.empty((B, C, N), np.float32)
    for core in range(8):
        bi, qh = core // 2, core % 2
        out[bi, :, qh * NQ:(qh + 1) * NQ] = results[core]["out"]
    return out.reshape(B, C, HW, HW)


def kernel(x, gn_scale, gn_bias, wq, bq, wk, bk, wv, bv, wp, bp):
    nc = _get_nc()
    in_maps = _make_in_maps(x, gn_scale, gn_bias, wq, bq, wk, bk, wv, bv,
                            wp, bp)
    res = run_bass_kernel_spmd(nc, in_maps, core_ids=list(range(8)))
    return _gather(res.results)


# revision 43
# speedup vs baseline: 1.5059x; 1.2916x over previous
"""Trainium2 Bass kernel for an AttnBlock (GroupNorm -> QKV 1x1 conv ->
spatial self-attention -> output projection -> residual).

Full-input contract: kernel(**inputs) takes the unsharded numpy inputs and
returns the full (4, 512, 64, 64) float32 output.

Sharding: 8 cores = 4 batches x 2 query-halves. Each core group-norms its
batch, runs attention for its 2048 queries over all 4096 keys, and writes
its query-half of the output. The per-core x input is column-rotated on the
host so that each core's own queries are always columns [0, 2048) — this
keeps the SPMD program identical across cores.

Algebraic fusions (all exact up to rounding):
- scores: q_i.k_j = h_j^T (Wk^T Wq) h_i + (Wk^T bq).h_j + [terms constant
  in j, dropped: softmax over j is invariant]. So K is never materialized;
  S = H^T @ R with R = W3^T H + w2, W3 = Wq^T Wk, w2 = Wk^T bq — adding w2
  to every column of R bakes the per-key bias into the score matmul.
- attention output: Wp @ (V P) = (Wp Wv) @ (H P) + Wp bv (softmax weights
  sum to 1), so V is never materialized either, and the output projection
  collapses into M2 = Wp Wv, plus w4 = Wp bv + bp.
- W3, M2^T, w2 and w4 are precomputed on the HOST (f32 numpy) and shipped
  pre-quantized: W3/M2T scaled by 16 into fp8e4m3 (their entries are
  ~N(0, 1/c), x16 centers them in fp8 range), w2/w4 in f32. This removes
  all on-chip weight prep and 4MB/core of weight DMA. A bf16 copy of x is
  also shipped for the GroupNorm/attention path (halves the critical input
  stream); the residual still adds the f32 x.
- softmax skips the max-subtraction; a constant -4.0 folded into the exp
  bias guards fp8e4m3 overflow (cancels exactly in the normalization).
  Denominators come straight from the PE: an all-ones fp8 DoubleRow matmul
  accumulates sum_j e2[j, i] alongside the attention matmuls (bit-identical
  to summing the same fp8 e2 tiles on DVE, but ~10x cheaper).

Numerics: h is stored fp8e4m3 only (GroupNorm output is ~N(0,1)); every
main-loop matmul (scores, R, attention-value, projection) runs in fp8 with
perf_mode=DoubleRow (two contraction sub-rows per PE cell, K=256 per
matmul); everything accumulates in fp32 PSUM. The attention accumulator is
divided by the softmax denominator BEFORE the output projection, so its
values live in the convex hull of h and re-quantize safely to fp8.
"""

from contextlib import ExitStack

import numpy as np

import concourse.mybir as mybir
import concourse.tile as tile
from concourse import bacc
from concourse.bass_utils import run_bass_kernel_spmd

# Problem geometry (hardcoded; the grading harness stages only kernel.py).
B = 4
C = 512
HW = 64
N = HW * HW          # 4096 keys per batch
NQ = N // 2          # 2048 queries per core
GROUPS = 32
GSIZE = C // GROUPS  # 16 channels per group
EPS = 1e-6
WSCALE = 16.0        # host-side scale on W3 / M2T before fp8 quantization

P = 128
CT = C // P          # 4 channel chunks
CP = CT // 2         # 2 channel chunk-pairs (fp8 DoubleRow)
JT = N // P          # 32 key chunks of 128
NI = 512             # free-dim tile (queries / keys / channels)
IC = NQ // NI        # 4 query chunks per core

F32 = mybir.dt.float32
BF16 = mybir.dt.bfloat16
F8 = mybir.dt.float8e4

PARAM_NAMES = ("gn_scale", "gn_bias", "w4")

_BUILD_CACHE = {}


def _emit(ctx, nc, tc, x_d, x16_d, w3_d, m2_d, w2_d, p_d, out_d, repeat=1):
    AF = mybir.ActivationFunctionType
    ALU = mybir.AluOpType

    consts = ctx.enter_context(tc.tile_pool(name="consts", bufs=1))
    small = ctx.enter_context(tc.tile_pool(name="small", bufs=4))
    stage = ctx.enter_context(tc.tile_pool(name="stage", bufs=2))
    big = ctx.enter_context(tc.tile_pool(name="big", bufs=2))
    rpool = ctx.enter_context(tc.tile_pool(name="rpool", bufs=1))
    wpool = ctx.enter_context(tc.tile_pool(name="wpool", bufs=1))
    epool = ctx.enter_context(tc.tile_pool(name="epool", bufs=6))
    attn_pool = ctx.enter_context(tc.tile_pool(name="attn_pool", bufs=2))
    outs_pool = ctx.enter_context(tc.tile_pool(name="outs_pool", bufs=3))
    mm_ps = ctx.enter_context(tc.tile_pool(name="mm_ps", bufs=3, space="PSUM"))
    acc_ps = ctx.enter_context(tc.tile_pool(name="acc_ps", bufs=4, space="PSUM"))

    for _rep in range(repeat):
        _emit_body(nc, tc, x_d, x16_d, w3_d, m2_d, w2_d, p_d, out_d, consts,
                   small, stage, big, rpool, wpool, epool, attn_pool,
                   outs_pool, mm_ps, acc_ps, AF, ALU, _rep)


def _emit_body(nc, tc, x_d, x16_d, w3_d, m2_d, w2_d, p_d, out_d, consts,
               small, stage, big, rpool, wpool, epool, attn_pool, outs_pool,
               mm_ps, acc_ps, AF, ALU, rep):
    # ---- constants -------------------------------------------------------
    # Pool-engine identity first (bf16, proven path), then a one-time DVE
    # cast to the fp8 identity the h-transposes use.
    ident_bf = consts.tile([P, P], BF16, tag="ident_bf")
    nc.gpsimd.memset(ident_bf, 0.0)
    nc.gpsimd.affine_select(
        out=ident_bf, in_=ident_bf, compare_op=ALU.not_equal, fill=1.0,
        base=0, pattern=[[-1, P]], channel_multiplier=1,
    )
    ident_f8 = consts.tile([P, P], F8, tag="ident_f8")
    nc.vector.tensor_copy(out=ident_f8, in_=ident_bf)
    # all-ones fp8 DoubleRow operand for the softmax denominators
    ones8 = consts.tile([P, 2, P], F8, tag="ones8")
    nc.vector.memset(ones8, 1.0)

    # Per-channel params as (128, CT): column cc = channels [cc*128, ..+128).
    # SWDGE (gpsimd) keeps these small gathers off the HWDGE queue that
    # streams x.
    par = {}
    for name in PARAM_NAMES:
        t = consts.tile([P, CT], F32, tag=f"par_{name}", name=f"par_{name}")
        nc.gpsimd.dma_start(out=t, in_=p_d[name][:].rearrange("(t p) -> p t", p=P))
        par[name] = t
    w2t = consts.tile([P, CT], F32, tag="w2t")
    nc.gpsimd.dma_start(out=w2t, in_=w2_d[:].rearrange("(t p) -> p t", p=P))
    # Host-fused weights (already x16 fp8). Also on SWDGE.
    w3t = wpool.tile([P, CT, C], F8, tag="w3t")
    nc.gpsimd.dma_start(out=w3t, in_=w3_d[:].rearrange("(t p) c -> p t c", p=P))
    m2t = wpool.tile([P, CT, C], F8, tag="m2t")
    nc.gpsimd.dma_start(out=m2t, in_=m2_d[:].rearrange("(t p) c -> p t c", p=P))

    # Group-reduction matrices. G: (128, 8) with G[p, g] = 1/GSIZE iff
    # p // GSIZE == g. GE: (8, 128) with GE[g, p] = 1 iff p // GSIZE == g.
    GPC = P // GSIZE  # 8 groups per 128-channel chunk
    gmat = consts.tile([P, GPC], F32, tag="gmat")
    nc.gpsimd.memset(gmat, 1.0 / GSIZE)
    nc.gpsimd.affine_select(
        out=gmat, in_=gmat, compare_op=ALU.is_ge, fill=0.0,
        base=0, pattern=[[-GSIZE, GPC]], channel_multiplier=1,
    )
    nc.gpsimd.affine_select(
        out=gmat, in_=gmat, compare_op=ALU.is_ge, fill=0.0,
        base=GSIZE - 1, pattern=[[GSIZE, GPC]], channel_multiplier=-1,
    )
    gexp = consts.tile([GPC, P], F32, tag="gexp")
    nc.gpsimd.memset(gexp, 1.0)
    nc.gpsimd.affine_select(
        out=gexp, in_=gexp, compare_op=ALU.is_ge, fill=0.0,
        base=0, pattern=[[1, P]], channel_multiplier=-GSIZE,
    )
    nc.gpsimd.affine_select(
        out=gexp, in_=gexp, compare_op=ALU.is_ge, fill=0.0,
        base=GSIZE - 1, pattern=[[-1, P]], channel_multiplier=GSIZE,
    )
    eps8 = consts.tile([GPC, 1], F32, tag="eps8")
    nc.vector.memset(eps8, EPS)
    neg4 = consts.tile([P, 1], F32, tag="neg4")
    nc.vector.memset(neg4, -4.0)

    # ---- x load + GroupNorm + normalize (straight to fp8 h) --------------
    h8 = big.tile([P, CT, N], F8, tag="big")
    # hT blocks (keys on partitions), filled per channel chunk as h lands.
    ht = big.tile([P, JT, C], F8, tag="big")
    for cc in range(CT):
        stats = small.tile([P, 8, 6], F32, tag="gn_stats",
                           name=f"gn_stats_{rep}_{cc}")
        # GroupNorm reads the host-provided bf16 copy of x (half the DMA
        # bytes; the f32 original is still used for the residual). Two
        # half-chunk DMAs so the stats for the first half overlap the
        # second half's transfer.
        xs = stage.tile([P, N], BF16, tag="xstage", name=f"xs_{rep}_{cc}",
                        bufs=4)
        for hh in range(2):
            nc.sync.dma_start(
                out=xs[:, hh * (N // 2):(hh + 1) * (N // 2)],
                in_=x16_d[cc * P:(cc + 1) * P,
                          hh * (N // 2):(hh + 1) * (N // 2)])
            for sg in range(4):
                s4 = hh * 4 + sg
                nc.vector.bn_stats(out=stats[:, s4, :],
                                   in_=xs[:, s4 * NI:(s4 + 1) * NI])
        mv = small.tile([P, 2], F32, tag="gn_mv")
        nc.vector.bn_aggr(out=mv, in_=stats)
        # stat2 = [mean_c, E[x^2]_c];  E[x^2] = mean^2 + var in one op
        stat2 = small.tile([P, 2], F32, tag="gn_stat2")
        nc.vector.tensor_copy(out=stat2[:, 0:1], in_=mv[:, 0:1])
        nc.vector.tensor_scalar(
            out=stat2[:, 1:2], in0=mv[:, 0:1], scalar1=mv[:, 0:1],
            scalar2=mv[:, 1:2], op0=ALU.mult, op1=ALU.add)
        # group-combine on PE: (8, 2) = G^T @ stat2
        g_ps = acc_ps.tile([GPC, 2], F32, tag="acc")
        nc.tensor.matmul(g_ps, lhsT=gmat, rhs=stat2, start=True, stop=True)
        g_sb = small.tile([GPC, 2], F32, tag="gn_gsb")
        nc.vector.tensor_copy(out=g_sb, in_=g_ps)
        # grp = [mean_g, rstd_g];  rstd via sqrt(-1*(mean^2 - E2) + eps)
        grp = small.tile([GPC, 2], F32, tag="gn_grp")
        nc.vector.tensor_copy(out=grp[:, 0:1], in_=g_sb[:, 0:1])
        nvar = small.tile([GPC, 1], F32, tag="gn_nvar")
        nc.vector.tensor_scalar(
            out=nvar, in0=g_sb[:, 0:1], scalar1=g_sb[:, 0:1],
            scalar2=g_sb[:, 1:2], op0=ALU.mult, op1=ALU.subtract)
        sd = small.tile([GPC, 1], F32, tag="gn_sd")
        nc.scalar.activation(out=sd, in_=nvar, func=AF.Sqrt, bias=eps8,
                             scale=-1.0)
        nc.vector.reciprocal(out=grp[:, 1:2], in_=sd)
        # expand back to per-channel via PE: (128, 2) = GE^T @ grp
        e_ps = acc_ps.tile([P, 2], F32, tag="acc")
        nc.tensor.matmul(e_ps, lhsT=gexp, rhs=grp, start=True, stop=True)
        e_sb = small.tile([P, 2], F32, tag="gn_esb")
        nc.vector.tensor_copy(out=e_sb, in_=e_ps)
        # a_c = gn_scale * rstd ; b_c = gn_bias - mean * a_c
        a_c = small.tile([P, 1], F32, tag="gn_a")
        nc.vector.tensor_mul(out=a_c, in0=par["gn_scale"][:, cc:cc + 1],
                             in1=e_sb[:, 1:2])
        nb_c = small.tile([P, 1], F32, tag="gn_nb")
        nc.vector.tensor_scalar(
            out=nb_c, in0=e_sb[:, 0:1], scalar1=a_c,
            scalar2=par["gn_bias"][:, cc:cc + 1],
            op0=ALU.mult, op1=ALU.subtract)
        b_c = small.tile([P, 1], F32, tag="gn_b")
        nc.vector.tensor_scalar_mul(out=b_c, in0=nb_c, scalar1=-1.0)
        # h8 = a_c * x + b_c on ACT (DVE keeps the stats work). The last
        # chunk is normalized in two pieces so R / the first scores can
        # start as soon as the leading columns are ready.
        if cc < CT - 1:
            nc.scalar.activation(
                out=h8[:, cc, :], in_=xs, func=AF.Identity,
                scale=a_c, bias=b_c)
        else:
            nc.scalar.activation(
                out=h8[:, cc, :2 * NI], in_=xs[:, :2 * NI],
                func=AF.Identity, scale=a_c, bias=b_c)
            last_norm = (xs, a_c, b_c)

    inv_sqrt_c = float(C) ** -0.5

    def emit_transposes(cc, dve_only, jgs=None):
        # hT blocks for channel chunk cc: 4 fp8 transposes packed per PSUM
        # bank (disjoint column ranges), one strided eviction per pack.
        # FP8 transpose writes with an element step of 2 in PSUM, so the
        # pack is allocated 2x wide and accessed with stride 2.
        for jg in (range(JT // 4) if jgs is None else jgs):
            tp = acc_ps.tile([P, 4, 2 * P], F8, tag="acc",
                             name=f"htp_{rep}_{cc}_{jg}")
            for k in range(4):
                jc = jg * 4 + k
                nc.tensor.matmul(
                    tp[:, k, 0:2 * P:2], lhsT=h8[:, cc, jc * P:(jc + 1) * P],
                    rhs=ident_f8, is_transpose=True, skip_group_check=True)
            dst = ht[:, jg * 4:(jg + 1) * 4, cc * P:(cc + 1) * P]
            src = tp[:, :, 0:2 * P:2]
            # GPSIMD cannot read PSUM on HW, so evictions split ACT/DVE:
            # ACT-heavy for the GroupNorm chunks (DVE paces the stats),
            # DVE-only for the last chunk (ACT must start the exps).
            if dve_only:
                eng = "dve"
            else:
                eng = ("act", "act", "dve", "act")[jg % 4]
            if eng == "dve":
                nc.vector.tensor_copy(out=dst, in_=src)
            else:
                nc.scalar.activation(out=dst, in_=src, func=AF.Identity)

    # R = W3^T-weighted H_q: R[a, i] = sum_b W3[b, a] h[b, i]; fp8
    # DoubleRow over b chunk-pairs, evicted /16 back to fp8 natural scale.
    r8 = rpool.tile([P, CT, NQ], F8, tag="r")

    def emit_r2(icq, ats):
        for at in ats:
            ps = mm_ps.tile([P, NI], F32, tag="mm",
                            name=f"rps_{rep}_{icq}_{at}")
            for bp_ in range(CP):
                nc.tensor.matmul(
                    ps, lhsT=w3t[:, 2 * bp_:2 * bp_ + 2, at * P:(at + 1) * P],
                    rhs=h8[:, 2 * bp_:2 * bp_ + 2, icq * NI:(icq + 1) * NI],
                    start=(bp_ == 0), stop=(bp_ == CP - 1),
                    perf_mode=mybir.MatmulPerfMode.DoubleRow)
            # The (Wk^T bq).h_j score-bias folds into R: adding w2 (a
            # per-partition constant across query columns) to every r8
            # column makes the score matmul emit s[j,i] + w2.h_j directly,
            # so the exp bias collapses to the constant -4.0 guard.
            nc.vector.tensor_scalar(
                out=r8[:, at, icq * NI:(icq + 1) * NI], in0=ps,
                scalar1=1.0 / WSCALE, scalar2=w2t[:, at:at + 1],
                op0=ALU.mult, op1=ALU.add)

    # Transposes for chunks 0..CT-2 already interleave with GroupNorm above;
    # for the last chunk, R(icq0) and the first hT blocks are emitted first
    # so the score pipeline can start as soon as h8's leading columns land.
    for cc in range(CT - 1):
        emit_transposes(cc, dve_only=False)
    emit_r2(0, range(CT))
    emit_transposes(CT - 1, dve_only=True, jgs=[0, 1])
    xs3, a_c3, b_c3 = last_norm
    nc.scalar.activation(
        out=h8[:, CT - 1, 2 * NI:], in_=xs3[:, 2 * NI:],
        func=AF.Identity, scale=a_c3, bias=b_c3)
    emit_transposes(CT - 1, dve_only=True, jgs=[2, 3, 4, 5, 6, 7])

    # ---- attention + output projection + residual ------------------------
    # Software-pipelined: the att/den matmuls trail the score matmuls by
    # ATT_LAG jp-steps so an in-order PE never head-blocks the ACT exp
    # stream; the epilogue of icq-1 is emitted inside icq's jp loop.
    ATT_LAG = 2
    JP = JT // 2

    def emit_attden(icq, att_ps, den_ps, jp, e2):
        # den first: it releases the reciprocal in the epilogue chain
        nc.tensor.matmul(
            den_ps, lhsT=ones8, rhs=e2,
            start=(jp == 0), stop=(jp == JP - 1),
            perf_mode=mybir.MatmulPerfMode.DoubleRow)
        for ct in range(CT):
            nc.tensor.matmul(
                att_ps[ct], lhsT=ht[:, 2 * jp:2 * jp + 2,
                                    ct * P:(ct + 1) * P],
                rhs=e2, start=(jp == 0), stop=(jp == JP - 1),
                perf_mode=mybir.MatmulPerfMode.DoubleRow)

    def emit_rec_evict(icq, att_ps, den_ps, rec, att8, cts, hs, do_rec):
        # rec + fp8 attention eviction, DVE/Pool alternated. Values land in
        # the convex hull of h (|.| <~ 5), safe for fp8.
        if do_rec:
            nc.vector.reciprocal(out=rec[:, hs], in_=den_ps[:, hs])
        for ct in cts:
            nc.vector.tensor_mul(out=att8[:, ct, hs],
                                 in0=att_ps[ct][:, hs], in1=rec[:, hs])

    def emit_proj(icq, att8, xr4, dc, hs, sp):
        NH = hs.stop - hs.start
        pp = mm_ps.tile([P, NH], F32, tag="mm",
                        name=f"pp_{rep}_{icq}_{sp}_{dc}")
        for ep_ in range(CP):
            nc.tensor.matmul(
                pp, lhsT=m2t[:, 2 * ep_:2 * ep_ + 2, dc * P:(dc + 1) * P],
                rhs=att8[:, 2 * ep_:2 * ep_ + 2, hs],
                start=(ep_ == 0), stop=(ep_ == CP - 1),
                perf_mode=mybir.MatmulPerfMode.DoubleRow)
        ob = outs_pool.tile([P, NH], F32, tag="ob",
                            name=f"ob_{rep}_{icq}_{sp}_{dc}")
        nc.vector.scalar_tensor_tensor(
            out=ob, in0=pp, scalar=1.0 / WSCALE,
            in1=xr4[:, dc, hs], op0=ALU.mult, op1=ALU.add)
        nc.sync.dma_start(
            out=out_d[dc * P:(dc + 1) * P,
                      icq * NI + hs.start:icq * NI + hs.stop],
            in_=ob)

    prev = None
    pst = None  # (rec, att8) of the in-flight epilogue
    lagq = []  # att/den matmuls trailing the score stream, across icq too
    FULL = slice(0, NI)
    for icq in range(IC):
        att_ps = [acc_ps.tile([P, NI], F32, tag="acc",
                              name=f"att_ps_{rep}_{icq}_{ct}")
                  for ct in range(CT)]
        den_ps = acc_ps.tile([P, NI], F32, tag="den", bufs=1,
                             name=f"den_ps_{rep}_{icq}")
        xr4 = None
        for jp in range(JP):
            # the previous query-chunk's epilogue, sliced thin across jp
            # slots so the injected PE work never outpaces the exp stream
            if prev is not None and 2 <= jp <= 8:
                picq, patt, pden, pxr4 = prev
                if jp == 2:
                    rec = outs_pool.tile([P, NI], F32, tag="rec", bufs=2,
                                         name=f"rec_{rep}_{picq}")
                    att8 = attn_pool.tile([P, CT, NI], F8, tag="attn",
                                          name=f"att8_{rep}_{picq}")
                    pst = (rec, att8)
                    emit_rec_evict(picq, patt, pden, rec, att8, (0, 1),
                                   FULL, do_rec=True)
                elif jp == 3:
                    emit_rec_evict(picq, patt, pden, pst[0], pst[1], (2, 3),
                                   FULL, do_rec=False)
                    if icq + 1 < IC:
                        emit_r2(icq + 1, (0, 1))
                elif jp == 4:
                    if icq + 1 < IC:
                        emit_r2(icq + 1, (2, 3))
                elif jp >= 5:
                    emit_proj(picq, pst[1], pxr4, jp - 5, FULL, 0)
                    if jp == 8:
                        prev = None
            elif prev is None and icq + 1 < IC and jp == 3:
                emit_r2(icq + 1, (0, 1))
            elif prev is None and icq + 1 < IC and jp == 4:
                emit_r2(icq + 1, (2, 3))
            if jp == 12:
                # residual load, emitted late so the scheduler cannot hoist
                # it ahead of the x chunk DMAs; w4 is folded in on Pool.
                xr = outs_pool.tile([P, CT, NI], F32, tag="xres", bufs=2,
                                    name=f"xr_{rep}_{icq}")
                nc.sync.dma_start(
                    out=xr, in_=x_d[:, icq * NI:(icq + 1) * NI].rearrange(
                        "(t p) n -> p t n", p=P))
                xr4 = outs_pool.tile([P, CT, NI], F32, tag="xres4", bufs=2,
                                     name=f"xr4_{rep}_{icq}")
                for dc in range(CT):
                    nc.gpsimd.tensor_scalar_add(
                        out=xr4[:, dc, :], in0=xr[:, dc, :],
                        scalar1=par["w4"][:, dc:dc + 1])
            e2 = epool.tile([P, 2, NI], F8, tag="e",
                            name=f"e2_{rep}_{icq}_{jp}")
            for half in range(2):
                jc = jp * 2 + half
                s_ps = mm_ps.tile([P, NI], F32, tag="mm",
                                  name=f"s_ps_{rep}_{icq}_{jc}")
                for ap_ in range(CP):
                    nc.tensor.matmul(
                        s_ps,
                        lhsT=h8[:, 2 * ap_:2 * ap_ + 2, jc * P:(jc + 1) * P],
                        rhs=r8[:, 2 * ap_:2 * ap_ + 2,
                               icq * NI:(icq + 1) * NI],
                        start=(ap_ == 0), stop=(ap_ == CP - 1),
                        perf_mode=mybir.MatmulPerfMode.DoubleRow)
                # -4.0 guards fp8e4m3 exp overflow; the e^-4 factor cancels
                # exactly in the softmax normalization.
                nc.scalar.activation(out=e2[:, half, :], in_=s_ps,
                                     func=AF.Exp, scale=inv_sqrt_c,
                                     bias=neg4)
            lagq.append((icq, att_ps, den_ps, jp, e2))
            if len(lagq) > ATT_LAG:
                emit_attden(*lagq.pop(0))
        prev = (icq, att_ps, den_ps, xr4)
    while lagq:
        emit_attden(*lagq.pop(0))
    # Tail epilogue, two column halves, chain spread over three engines:
    # ACT evicts the raw accumulator scaled by a CONSTANT 1/256 (fits fp8
    # without the rec round-trip), DVE applies 16*rec after the projection,
    # Pool adds the residual (SBUF-only, legal for GPSIMD).
    licq, latt, lden, lxr4 = prev
    rec = outs_pool.tile([P, NI], F32, tag="rec", bufs=2,
                         name=f"rec_{rep}_{licq}")
    att8 = attn_pool.tile([P, CT, NI], F8, tag="attn",
                          name=f"att8_{rep}_{licq}")
    ob4s = [outs_pool.tile([P, CT, NI // 2], F32, tag="ob4", bufs=2,
                           name=f"ob4_{rep}_{licq}_{sp}")
            for sp in range(2)]
    for sp in range(2):
        NH = NI // 2
        hs = slice(sp * NH, (sp + 1) * NH)
        nc.vector.reciprocal(out=rec[:, hs], in_=lden[:, hs])
        for ct in range(CT):
            nc.scalar.activation(out=att8[:, ct, hs], in_=latt[ct][:, hs],
                                 func=AF.Identity, scale=1.0 / 256.0)
        for dc in range(CT):
            pp = mm_ps.tile([P, NH], F32, tag="mm",
                            name=f"pp_{rep}_{licq}_{sp}_{dc}")
            for ep_ in range(CP):
                nc.tensor.matmul(
                    pp, lhsT=m2t[:, 2 * ep_:2 * ep_ + 2, dc * P:(dc + 1) * P],
                    rhs=att8[:, 2 * ep_:2 * ep_ + 2, hs],
                    start=(ep_ == 0), stop=(ep_ == CP - 1),
                    perf_mode=mybir.MatmulPerfMode.DoubleRow)
            tnorm = outs_pool.tile([P, NH], F32, tag="tnorm", bufs=4,
                                   name=f"tn_{rep}_{licq}_{sp}_{dc}")
            nc.vector.scalar_tensor_tensor(
                out=tnorm, in0=pp, scalar=256.0 / WSCALE, in1=rec[:, hs],
                op0=ALU.mult, op1=ALU.mult)
            ob4 = ob4s[sp]
            nc.gpsimd.tensor_add(out=ob4[:, dc, :], in0=tnorm,
                                 in1=lxr4[:, dc, hs])
        # one batched DMA per column half instead of four small ones
        nc.sync.dma_start(
            out=out_d[:, licq * NI + hs.start:licq * NI + hs.stop].rearrange(
                "(t p) n -> p t n", p=P),
            in_=ob4)


def _build(repeat=1):
    nc = bacc.Bacc()
    x_d = nc.declare_dram_parameter("x", [C, N], F32, isOutput=False)
    x16_d = nc.declare_dram_parameter("x16", [C, N], BF16, isOutput=False)
    w3_d = nc.declare_dram_parameter("w3", [C, C], F8, isOutput=False)
    m2_d = nc.declare_dram_parameter("m2t", [C, C], F8, isOutput=False)
    w2_d = nc.declare_dram_parameter("w2", [C], F32, isOutput=False)
    p_d = {p: nc.declare_dram_parameter(p, [C], F32, isOutput=False)
           for p in PARAM_NAMES}
    out_d = nc.declare_dram_parameter("out", [C, NQ], F32, isOutput=True)
    with tile.TileContext(nc) as tc, ExitStack() as ctx:
        _emit(ctx, nc, tc, x_d, x16_d, w3_d, m2_d, w2_d, p_d, out_d,
              repeat=repeat)
    nc.finalize()
    return nc


def _get_nc():
    if "nc" not in _BUILD_CACHE:
        _BUILD_CACHE["nc"] = _build()
    return _BUILD_CACHE["nc"]


def _make_in_maps(x, gn_scale, gn_bias, wq, bq, wk, bk, wv, bv, wp, bp):
    f8np = mybir.dt.np(F8)
    xf = np.ascontiguousarray(np.asarray(x, dtype=np.float32).reshape(B, C, N))
    wqf = np.asarray(wq, np.float32)
    wkf = np.asarray(wk, np.float32)
    wvf = np.asarray(wv, np.float32)
    wpf = np.asarray(wp, np.float32)
    w3f = (wqf.T @ wkf) * WSCALE
    m2f = (wpf @ wvf).T * WSCALE
    w2f = wkf.T @ np.asarray(bq, np.float32)
    w4f = wpf @ np.asarray(bv, np.float32) + np.asarray(bp, np.float32)
    shared = {
        "w3": np.ascontiguousarray(w3f.astype(f8np)),
        "m2t": np.ascontiguousarray(m2f.astype(f8np)),
        "w2": np.ascontiguousarray(w2f.astype(np.float32)),
        "w4": np.ascontiguousarray(w4f),
        "gn_scale": np.ascontiguousarray(np.asarray(gn_scale, np.float32)),
        "gn_bias": np.ascontiguousarray(np.asarray(gn_bias, np.float32)),
    }
    import ml_dtypes
    in_maps = []
    for core in range(8):
        bi, qh = core // 2, core % 2
        xb = xf[bi]
        if qh == 0:
            xc = xb
        else:
            xc = np.ascontiguousarray(
                np.concatenate([xb[:, NQ:], xb[:, :NQ]], axis=1))
        x16 = np.ascontiguousarray(xc.astype(ml_dtypes.bfloat16))
        in_maps.append({"x": xc, "x16": x16, **shared})
    return in_maps


def _gather(results):
    out = np.empty((B, C, N), np.float32)
    for core in range(8):
        bi, qh = core // 2, core % 2
        out[bi, :, qh * NQ:(qh + 1) * NQ] = results[core]["out"]
    return out.reshape(B, C, HW, HW)


def kernel(x, gn_scale, gn_bias, wq, bq, wk, bk, wv, bv, wp, bp):
    nc = _get_nc()
    in_maps = _make_in_maps(x, gn_scale, gn_bias, wq, bq, wk, bk, wv, bv,
                            wp, bp)
    res = run_bass_kernel_spmd(nc, in_maps, core_ids=list(range(8)))
    return _gather(res.results)
